# revision 21
# baseline (speedup 1.0000x reference)
"""Trainium2 Bass kernel for the dual-branch agent-attention module.

Sharding: data-parallel over B=8 (one batch element per NeuronCore).
All transposes and weight permutations are done host-side; on-device
work is a streamed pipeline with the two big score contractions run in
fp8e4m3 DoubleRow mode (4 rows of K per PE pass -> 3x fewer tensor
cycles than bf16), which is safe here because the effective score
weights are tiny (|s| < 0.7) and the softmax is near-uniform.

Structure:
  prep:  agent projections k_ag/qa -> block-diagonal tiles; effective
         score weights Weff_A = Wq @ k12bd and Weff_B = Wkhf @ qabd
         (the big activations never materialize q or kh), written as
         fp8 scaled by 64 (undone via the exp activation's scale).
  AC-1:  branch-A score maps pa = exp(x @ Weff_A / 64) for all six
         head pairs, computed via fp8-DR matmuls; interleaved between
         early phase-B tiles so the scalar engine's exps overlap
         tensor work.
  B:     v = attnT^T@Wv (ones col = softmax denominator), tT scores
         from attn8 via Weff_B (fp8-DR) -> exp -> xs accumulated
         directly in a persistent PSUM bank across all 32 seq tiles.
  AC-2:  x_out = pa^T @ xs_bd with ones-column denominators,
         normalize, PE-transpose, proj.

Head-major layout trick: the 2C projection outputs are permuted host-
side from (branch, head, d) to (head, branch, d) so each head pair
occupies one 128-partition tile; branch score scales (wa/wb * D^-0.5)
are folded into the K-side weights, so both branches' score maps come
out of a single contraction per head pair.

Bias handling: k-side biases that are constant along a softmax axis
cancel exactly and are dropped (bk_hf entirely; q-side bias of branch
A survives as the per-agent term c_A = k12bd^T @ bq, applied as the
exp's per-partition bias together with ba).
"""

import os
import sys
import numpy as np

for _p in ("/opt/trn_rl_repo", os.path.expanduser("~/.axon_site/_ro/trn_rl_repo")):
    if os.path.isdir(_p) and _p not in sys.path:
        sys.path.insert(0, _p)

import ml_dtypes

import concourse.bass as bass
import concourse.bacc as bacc
import concourse.tile as tile
from concourse import mybir
from concourse.bass_utils import run_bass_kernel_spmd

BF16 = mybir.dt.bfloat16
FP8 = mybir.dt.float8e4
F32 = mybir.dt.float32
NPBF16 = ml_dtypes.bfloat16
NPFP8 = ml_dtypes.float8_e4m3
DR = mybir.MatmulPerfMode.DoubleRow

B, N, NA, H, D = 8, 4096, 64, 12, 32
C = H * D            # 384
C2 = 2 * C           # 768
NP = H // 2          # 6 head pairs
NT = N // 128        # 32 seq tiles
SCALE = D ** -0.5
WSC = 64.0           # fp8 weight scale for the score contractions
ISC = 1.0 / WSC

_CACHE = {}


def _prune_ldweights(nc):
    """Drop InstLdweights whose weights AP matches the currently loaded one.

    The PE array keeps its stationary weights across matmuls; bass emits a
    fresh LDWEIGHTS per matmul regardless. Back-to-back matmuls that share
    lhsT (same AP, perf mode, tile geometry) only need the first load.
    Only bare loads (no semaphore waits/updates) are removed.
    """
    removed = 0
    for f in nc.m.functions:
        for bb in f.blocks:
            cur = None
            to_remove = []
            for inst in bb.instructions:
                tn = type(inst).__name__
                if getattr(inst, "engine", None) != mybir.EngineType.PE:
                    continue
                if tn == "InstLdweights":
                    ap = inst.ins[0]
                    sig = (ap.memref, ap.offset,
                           tuple(tuple(x) for x in ap.ap), str(ap.dtype),
                           str(getattr(inst, "perf_mode", None)),
                           bool(getattr(inst, "is_transpose", False)),
                           tuple(inst.tile_position) if inst.tile_position else None,
                           tuple(inst.tile_size) if inst.tile_size else None)
                    si = inst.sync_info
                    clean = si is None or (len(si.on_wait) == 0
                                           and len(si.on_update) == 0)
                    if cur == sig and clean:
                        to_remove.append(inst)
                    else:
                        cur = sig
                elif tn == "InstMatmult":
                    if getattr(inst, "is_transpose", False):
                        cur = None
                elif tn == "InstEventSemaphore":
                    pass
                else:
                    cur = None
            for inst in to_remove:
                bb.instructions.remove(inst)
                removed += 1
    return removed


def _build_bass(finalize=True, zero_bias=False, debug=False):
    nc = bacc.Bacc()

    # ---- DRAM I/O ----
    x8 = nc.dram_tensor("x8", [C, N], FP8, kind="ExternalInput")
    attn8 = nc.dram_tensor("attn8", [C, N], FP8, kind="ExternalInput")
    attnT = nc.dram_tensor("attnT", [C, N], BF16, kind="ExternalInput")
    agT = nc.dram_tensor("agT", [C, NA], BF16, kind="ExternalInput")
    wqT = nc.dram_tensor("wqT", [C2, C], BF16, kind="ExternalInput")
    wkag = nc.dram_tensor("wkag", [C, C2], BF16, kind="ExternalInput")
    wqag = nc.dram_tensor("wqag", [C, C2], BF16, kind="ExternalInput")
    wkhfT = nc.dram_tensor("wkhfT", [C2, C], BF16, kind="ExternalInput")
    wv = nc.dram_tensor("wv", [C, H * 33], BF16, kind="ExternalInput")
    wproj = nc.dram_tensor("wproj", [C, C], BF16, kind="ExternalInput")
    bq = nc.dram_tensor("bq", [C2], F32, kind="ExternalInput")
    bkag = nc.dram_tensor("bkag", [C2], F32, kind="ExternalInput")
    bqag = nc.dram_tensor("bqag", [C2], F32, kind="ExternalInput")
    bv = nc.dram_tensor("bv", [H * 33], F32, kind="ExternalInput")
    bproj = nc.dram_tensor("bproj", [C], F32, kind="ExternalInput")
    bab = nc.dram_tensor("bab", [2], F32, kind="ExternalInput")
    out = nc.dram_tensor("out", [N, C], F32, kind="ExternalOutput")
    if debug:
        d_pa = nc.dram_tensor("d_pa", [128, 6, N], BF16, kind="ExternalOutput")
        d_xsbd = nc.dram_tensor("d_xsbd", [128, 6 * 66], BF16, kind="ExternalOutput")
        d_pt0 = nc.dram_tensor("d_pt0", [128, C2], BF16, kind="ExternalOutput")
        d_v0 = nc.dram_tensor("d_v0", [128, H * 33], BF16, kind="ExternalOutput")
        d_weffa = nc.dram_tensor("d_weffa", [128, 4, C2], FP8, kind="ExternalOutput")
        d_weffb = nc.dram_tensor("d_weffb", [128, 4, C2], FP8, kind="ExternalOutput")
        d_pt5 = nc.dram_tensor("d_pt5", [128, C2], BF16, kind="ExternalOutput")
        d_v5 = nc.dram_tensor("d_v5", [128, H * 33], BF16, kind="ExternalOutput")
        d_xsraw = nc.dram_tensor("d_xsraw", [128, 6 * 66], F32, kind="ExternalOutput")

    Exp = mybir.ActivationFunctionType.Exp
    Copy = mybir.ActivationFunctionType.Copy

    with tile.TileContext(nc) as tc:
        with (
            tc.tile_pool(name="const", bufs=1) as const,
            tc.tile_pool(name="pt", bufs=2) as p_pt,
            tc.tile_pool(name="vv", bufs=2) as p_v,
            tc.tile_pool(name="xon", bufs=2) as p_xon,
            tc.tile_pool(name="xot", bufs=3) as p_xot,
            tc.tile_pool(name="osb", bufs=3) as p_out,
            tc.tile_pool(name="sm", bufs=4) as p_sm,
            tc.tile_pool(name="psA", bufs=2, space="PSUM") as psA,
            tc.tile_pool(name="psS", bufs=2, space="PSUM") as psS,
            tc.tile_pool(name="psV", bufs=1, space="PSUM") as psV,
            tc.tile_pool(name="psX", bufs=1, space="PSUM") as psX,
        ):
            # ---- small constants (gpsimd DMA queue); wproj last (AC-2 only)
            ag_t = const.tile([128, 3, NA], BF16)
            nc.gpsimd.dma_start(out=ag_t, in_=agT.rearrange("(k p) m -> p k m", p=128))
            w_kag = const.tile([128, 3, C2], BF16)
            w_qag = const.tile([128, 3, C2], BF16)
            w_qT = const.tile([128, 6, C], BF16)
            w_khfT = const.tile([128, 6, C], BF16)
            w_v = const.tile([128, 3, H * 33], BF16)
            w_pr = const.tile([128, 3, C], BF16)
            for dst, src in ((w_kag, wkag), (w_qag, wqag), (w_qT, wqT),
                             (w_khfT, wkhfT), (w_v, wv)):
                nc.gpsimd.dma_start(out=dst, in_=src.rearrange("(k p) m -> p k m", p=128))
            b_q = const.tile([128, 6], F32)
            b_kag = const.tile([128, 6], F32)
            b_qag = const.tile([128, 6], F32)
            for dst, src in ((b_q, bq), (b_kag, bkag), (b_qag, bqag)):
                nc.gpsimd.dma_start(out=dst, in_=src.rearrange("(j p) -> p j", p=128))
            bv_row = const.tile([1, H * 33], BF16)
            nc.gpsimd.dma_start(out=bv_row, in_=bv[:].unsqueeze(0))
            bpr_row = const.tile([1, C], BF16)
            nc.gpsimd.dma_start(out=bpr_row, in_=bproj[:].unsqueeze(0))
            ones_row = const.tile([1, 128], BF16)
            nc.vector.memset(ones_row, 1.0)
            ba_t = const.tile([128, 1], F32)
            nc.gpsimd.dma_start(out=ba_t, in_=bass.AP(tensor=bab[:].tensor, offset=0,
                                                      ap=[[0, 128], [1, 1]]))
            bb_t = const.tile([128, 1], F32)
            nc.gpsimd.dma_start(out=bb_t, in_=bass.AP(tensor=bab[:].tensor, offset=1,
                                                      ap=[[0, 128], [1, 1]]))
            nc.gpsimd.dma_start(out=w_pr, in_=wproj.rearrange("(k p) m -> p k m", p=128))

            # ---- big input streams, split across sync + scalar DMA queues,
            # small leading pieces so the first tiles unblock early ----
            x8_t = const.tile([128, 4, N], FP8)
            at8_t = const.tile([128, 4, N], FP8)
            att_t = const.tile([128, 3, N], BF16)
            nc.vector.memset(x8_t[:, 3, :], 0.0)
            nc.vector.memset(at8_t[:, 3, :], 0.0)
            x8r = x8.rearrange("(k p) s -> p k s", p=128)
            at8r = attn8.rearrange("(k p) s -> p k s", p=128)
            attr = attnT.rearrange("(k p) s -> p k s", p=128)
            bounds = (0, 256, 512, 1024, 2048, 3072, 4096)
            for lo, hi in zip(bounds[:-1], bounds[1:]):
                sl = slice(lo, hi)
                nc.sync.dma_start(out=at8_t[:, 0:3, sl], in_=at8r[:, :, sl])
                nc.scalar.dma_start(out=att_t[:, :, sl], in_=attr[:, :, sl])
                nc.sync.dma_start(out=x8_t[:, 0:3, sl], in_=x8r[:, :, sl])

            # Pre-touch small DMA constants so wide consumers only carry
            # the producing engine's wait.
            touch = const.tile([128, 16], F32)
            for i, t_ap in enumerate((b_q[:, 0:1], b_kag[:, 0:1], b_qag[:, 0:1],
                                      ba_t[:, 0:1], bb_t[:, 0:1])):
                nc.vector.tensor_copy(touch[:, i:i + 1], t_ap)
            nc.scalar.copy(touch[:, 8:9], ba_t[:, 0:1])
            nc.scalar.copy(touch[:, 9:10], bb_t[:, 0:1])

            # ---- prep: k_ag / qa projections -> block-diag tiles ----
            kag_sb = const.tile([128, 6, NA], BF16)
            qa_sb = const.tile([128, 6, NA], BF16)
            for w_t, b_t, dst in ((w_kag, b_kag, kag_sb), (w_qag, b_qag, qa_sb)):
                for j in range(6):
                    ps = psA.tile([128, NA], F32, tag="a")
                    for k in range(3):
                        nc.tensor.matmul(ps, lhsT=w_t[:, k, j * 128:(j + 1) * 128],
                                         rhs=ag_t[:, k, :], start=(k == 0), stop=(k == 2))
                    nc.vector.tensor_add(dst[:, j, :], ps,
                                         b_t[:, j:j + 1].to_broadcast([128, NA]))
            k12bd = const.tile([128, 6, 128], BF16)
            qabd = const.tile([128, 6, 128], BF16)
            for src, dst in ((kag_sb, k12bd), (qa_sb, qabd)):
                nc.vector.memset(dst, 0.0)
                for j in range(6):
                    nc.vector.tensor_copy(dst[0:64, j, 0:64], src[0:64, j, :])
                    nc.vector.tensor_copy(dst[64:128, j, 64:128], src[64:128, j, :])

            # ---- prep: effective score weights (fp8, scaled by WSC) ----
            weff_a8 = const.tile([128, 4, C2], FP8)
            weff_b8 = const.tile([128, 4, C2], FP8)
            nc.vector.memset(weff_a8[:, 3, :], 0.0)
            nc.vector.memset(weff_b8[:, 3, :], 0.0)
            for j in range(6):
                for k in range(3):
                    ps = psA.tile([128, 128], F32, tag="a")
                    nc.tensor.matmul(ps, lhsT=w_qT[:, j, k * 128:(k + 1) * 128],
                                     rhs=k12bd[:, j, :], start=True, stop=True)
                    nc.scalar.activation(weff_a8[:, k, j * 128:(j + 1) * 128], ps,
                                         Copy, scale=WSC)
                    ps2 = psA.tile([128, 128], F32, tag="a")
                    nc.tensor.matmul(ps2, lhsT=w_khfT[:, j, k * 128:(k + 1) * 128],
                                     rhs=qabd[:, j, :], start=True, stop=True)
                    nc.scalar.activation(weff_b8[:, k, j * 128:(j + 1) * 128], ps2,
                                         Copy, scale=WSC)
            cba = None
            if not zero_bias:
                b_q_bf = const.tile([128, 6], BF16)
                nc.vector.tensor_copy(b_q_bf, b_q)
                cba = const.tile([128, 6], F32)
                for j in range(6):
                    ps = psA.tile([128, 1], F32, tag="a")
                    nc.tensor.matmul(ps, lhsT=k12bd[:, j, :], rhs=b_q_bf[:, j:j + 1],
                                     start=True, stop=True)
                    nc.vector.tensor_add(cba[:, j:j + 1], ps, ba_t[:, 0:1])

            # ---- interleaved AC-1 (branch-A score maps) + phase B ----
            pa_t = const.tile([128, 6, N], BF16)
            xs_ps = psX.tile([128, 6 * 66], F32, tag="x")

            def ac1_unit(u):
                # one head pair x two seq chunks; shared lhsT back-to-back so
                # the redundant LDWEIGHTS prune below can drop half the loads
                j, cp = u // 4, u % 4
                tiles = []
                for ci in (2 * cp, 2 * cp + 1):
                    csl = slice(ci * 512, (ci + 1) * 512)
                    ps = psA.tile([128, 512], F32, tag="a", name=f"ac1_{u}_{ci}")
                    tiles.append((ps, csl))
                for kp in (0, 1):
                    for ps, csl in tiles:
                        nc.tensor.matmul(ps,
                                         lhsT=weff_a8[:, 2 * kp:2 * kp + 2,
                                                      j * 128:(j + 1) * 128],
                                         rhs=x8_t[:, 2 * kp:2 * kp + 2, csl],
                                         start=(kp == 0), stop=(kp == 1),
                                         perf_mode=DR)
                abias = 0.0 if zero_bias else cba[:, j:j + 1]
                for ps, csl in tiles:
                    nc.scalar.activation(pa_t[:, j, csl], ps, Exp, bias=abias,
                                         scale=ISC)

            for t in range(NT):
                if t < 24:
                    ac1_unit(t)
                tsl = slice(t * 128, (t + 1) * 128)
                # branch-B scores for this seq tile (fp8 DoubleRow)
                ps_s = psS.tile([128, C2], F32, tag="s")
                for kp in (0, 1):
                    for r0, r1 in ((0, 512), (512, 768)):
                        nc.tensor.matmul(ps_s[:, r0:r1],
                                         lhsT=at8_t[:, 2 * kp:2 * kp + 2, tsl],
                                         rhs=weff_b8[:, 2 * kp:2 * kp + 2, r0:r1],
                                         start=(kp == 0), stop=(kp == 1),
                                         perf_mode=DR)
                pt = p_pt.tile([128, C2], BF16)
                bbias = 0.0 if zero_bias else bb_t[:, 0:1]
                nc.scalar.activation(pt, ps_s, Exp, bias=bbias, scale=ISC)
                # v for this seq tile
                psv = psV.tile([128, H * 33], F32, tag="v")
                for k in range(3):
                    nc.tensor.matmul(psv, lhsT=att_t[:, k, tsl], rhs=w_v[:, k, :],
                                     start=(k == 0), stop=zero_bias and (k == 2))
                v_t = p_v.tile([128, H * 33], BF16)
                v3 = v_t[:].rearrange("p (h c) -> p h c", c=33)
                if zero_bias:
                    pv3 = psv.rearrange("p (h c) -> p h c", c=33)
                    nc.vector.tensor_copy(v3[:, :, 0:32], pv3[:, :, 0:32])
                    nc.vector.memset(v3[:, :, 32], 1.0)
                else:
                    nc.tensor.matmul(psv, lhsT=ones_row[:, :], rhs=bv_row[:, :],
                                     start=False, stop=True)
                    nc.vector.tensor_copy(v_t, psv)
                # xs accumulation directly in PSUM across all tiles
                for j in range(6):
                    # start only on the very first write: start_tensor_calc
                    # marks the whole 2KB bank pending-zero, so each later
                    # region's first write overwrites, then accumulates.
                    nc.tensor.matmul(xs_ps[:, j * 66:(j + 1) * 66],
                                     lhsT=pt[:, j * 128:(j + 1) * 128],
                                     rhs=v_t[:, j * 66:(j + 1) * 66],
                                     start=(t == 0 and j == 0), stop=(t == NT - 1),
                                     skip_group_check=True)
                if debug and t == 0:
                    nc.sync.dma_start(out=d_pt0[:, :], in_=pt)
                    nc.sync.dma_start(out=d_v0[:, :], in_=v_t)
                if debug and t == 5:
                    nc.sync.dma_start(out=d_pt5[:, :], in_=pt)
                    nc.sync.dma_start(out=d_v5[:, :], in_=v_t)

            # ---- xs normalize -> block-diag [xs | 1] tiles ----
            xs_bd = const.tile([128, 6 * 66], BF16)
            xs3 = xs_ps.rearrange("p (j c) -> p j c", c=66)
            bd3 = xs_bd[:].rearrange("p (j c) -> p j c", c=66)
            nc.vector.memset(xs_bd, 0.0)
            nc.vector.memset(bd3[0:64, :, 32:33], 1.0)
            nc.vector.memset(bd3[64:128, :, 65:66], 1.0)
            rec6 = p_sm.tile([128, 6], F32, tag="rec")
            nc.vector.reciprocal(rec6[0:64, :], xs3[0:64, :, 32])
            nc.vector.reciprocal(rec6[64:128, :], xs3[64:128, :, 65])
            nc.vector.tensor_mul(bd3[0:64, :, 0:32], xs3[0:64, :, 0:32],
                                 rec6[0:64, :].unsqueeze(2).to_broadcast([64, 6, 32]))
            nc.vector.tensor_mul(bd3[64:128, :, 33:65], xs3[64:128, :, 33:65],
                                 rec6[64:128, :].unsqueeze(2).to_broadcast([64, 6, 32]))

            if debug:
                xsr_sb = const.tile([128, 6 * 66], F32)
                nc.scalar.copy(xsr_sb, xs_ps)
                nc.sync.dma_start(out=d_xsraw[:, :], in_=xsr_sb)
                nc.sync.dma_start(out=d_pa[:, :, :], in_=pa_t)
                nc.sync.dma_start(out=d_xsbd[:, :], in_=xs_bd)
                nc.sync.dma_start(out=d_weffa[:, :, :], in_=weff_a8)
                nc.sync.dma_start(out=d_weffb[:, :, :], in_=weff_b8)

            # ---- AC-2: x_out, normalize, transpose, proj ----
            for t in range(NT):
                tsl = slice(t * 128, (t + 1) * 128)
                xo_ps = psA.tile([128, 12 * 33], F32, tag="a", name=f"xo_{t}")
                for j in range(6):
                    nc.tensor.matmul(xo_ps[:, j * 66:(j + 1) * 66],
                                     lhsT=pa_t[:, j, tsl],
                                     rhs=xs_bd[:, j * 66:(j + 1) * 66],
                                     start=True, stop=True)
                xo3 = xo_ps.rearrange("p (k c) -> p k c", c=33)
                rec = p_sm.tile([128, 12], F32, tag="rec12")
                nc.vector.reciprocal(rec, xo3[:, :, 32])
                xon = p_xon.tile([128, C], BF16)
                nc.vector.tensor_mul(xon[:].rearrange("p (k c) -> p k c", c=32),
                                     xo3[:, :, 0:32],
                                     rec[:].unsqueeze(2).to_broadcast([128, 12, 32]))
                xot = p_xot.tile([128, 3, 128], BF16)
                nc.scalar.dma_start_transpose(out=xot, in_=xon)
                pr_ps = psS.tile([128, C], F32, tag="s", name=f"pr_{t}")
                for f in range(3):
                    nc.tensor.matmul(pr_ps, lhsT=xot[:, f, :],
                                     rhs=w_pr[:, f, :],
                                     start=(f == 0),
                                     stop=(zero_bias and f == 2),
                                     skip_group_check=True)
                if not zero_bias:
                    nc.tensor.matmul(pr_ps, lhsT=ones_row[:, :], rhs=bpr_row[:, :],
                                     start=False, stop=True, skip_group_check=True)
                o_sb = p_out.tile([128, C], F32)
                nc.scalar.copy(o_sb, pr_ps)
                nc.sync.dma_start(out=out[t * 128:(t + 1) * 128, :], in_=o_sb)
    _prune_ldweights(nc)
    if finalize:
        nc.finalize()
    return nc


def _prep_host(inputs):
    f32 = np.float32
    x = np.asarray(inputs["x"], f32)
    attn = np.asarray(inputs["attn"], f32)
    agent = np.asarray(inputs["agent_input"], f32)
    wa = np.asarray(inputs["wa"], f32)
    wb = np.asarray(inputs["wb"], f32)

    perm = np.empty(C2, np.int64)
    sva = np.empty(C2, f32)
    svb = np.empty(C2, f32)
    for h in range(H):
        for br in range(2):
            j0 = h * 64 + br * 32
            perm[j0:j0 + 32] = br * C + h * 32 + np.arange(32)
            sva[j0:j0 + 32] = wa[br] * SCALE
            svb[j0:j0 + 32] = wb[br] * SCALE

    wq_p = np.asarray(inputs["Wq_lf"], f32)[:, perm]
    bq_p = np.asarray(inputs["bq_lf"], f32)[perm]
    wkag_p = np.asarray(inputs["Wk_ag"], f32)[:, perm] * sva[None, :]
    bkag_p = np.asarray(inputs["bk_ag"], f32)[perm] * sva
    wqag_p = np.asarray(inputs["Wq_ag"], f32)[:, perm]
    bqag_p = np.asarray(inputs["bq_ag"], f32)[perm]
    wkhf_p = np.asarray(inputs["Wk_hf"], f32)[:, perm] * svb[None, :]

    wv_in = np.asarray(inputs["Wv_hf"], f32)
    bv_in = np.asarray(inputs["bv_hf"], f32)
    wv_aug = np.zeros((C, H * 33), f32)
    bv_aug = np.zeros(H * 33, f32)
    for h in range(H):
        wv_aug[:, h * 33:h * 33 + 32] = wv_in[:, h * 32:h * 32 + 32]
        bv_aug[h * 33:h * 33 + 32] = bv_in[h * 32:h * 32 + 32]
        bv_aug[h * 33 + 32] = 1.0

    bab = np.array([np.asarray(inputs["ba"], f32)[0],
                    np.asarray(inputs["bb"], f32)[0]], f32)

    shared = {
        "wqT": np.ascontiguousarray(wq_p.T).astype(NPBF16),
        "wkhfT": np.ascontiguousarray(wkhf_p.T).astype(NPBF16),
        "wkag": wkag_p.astype(NPBF16),
        "wqag": wqag_p.astype(NPBF16),
        "wv": wv_aug.astype(NPBF16),
        "wproj": np.asarray(inputs["Wproj"], f32).astype(NPBF16),
        "bq": bq_p, "bkag": bkag_p, "bqag": bqag_p,
        "bv": bv_aug, "bproj": np.ascontiguousarray(np.asarray(inputs["bproj"], f32)),
        "bab": bab,
    }
    xT = np.ascontiguousarray(x.transpose(0, 2, 1))
    attnT = np.ascontiguousarray(attn.transpose(0, 2, 1))
    agT = np.ascontiguousarray(agent.transpose(0, 2, 1)).astype(NPBF16)
    x8 = xT.astype(NPFP8)
    attn8 = attnT.astype(NPFP8)
    attnTb = attnT.astype(NPBF16)
    in_maps = []
    for b in range(B):
        m = dict(shared)
        m["x8"] = x8[b]
        m["attn8"] = attn8[b]
        m["attnT"] = attnTb[b]
        m["agT"] = agT[b]
        in_maps.append(m)
    return in_maps


def kernel(**inputs):
    zb = all(not np.any(np.asarray(inputs[k]))
             for k in ("bq_lf", "bk_ag", "bq_ag", "bk_hf", "bv_hf", "bproj",
                       "ba", "bb"))
    key = ("nc", zb)
    if key not in _CACHE:
        _CACHE[key] = _build_bass(zero_bias=zb)
    nc = _CACHE[key]
    in_maps = _prep_host(inputs)
    res = run_bass_kernel_spmd(nc, in_maps, core_ids=list(range(B)))
    return np.stack([res.results[b]["out"] for b in range(B)], axis=0)


# revision 23
# speedup vs baseline: 1.1031x; 1.1031x over previous
"""Trainium2 Bass kernel for the dual-branch agent-attention module.

Sharding: data-parallel over B=8 (one batch element per NeuronCore).
All transposes and weight permutations are done host-side; on-device
work is a streamed pipeline with the two big score contractions run in
fp8e4m3 DoubleRow mode (4 rows of K per PE pass -> 3x fewer tensor
cycles than bf16), which is safe here because the effective score
weights are tiny (|s| < 0.7) and the softmax is near-uniform.

Structure:
  prep:  agent projections k_ag/qa -> block-diagonal tiles; effective
         score weights Weff_A = Wq @ k12bd and Weff_B = Wkhf @ qabd
         (the big activations never materialize q or kh), written as
         fp8 scaled by 64 (undone via the exp activation's scale).
  AC-1:  branch-A score maps pa = exp(x @ Weff_A / 64) for all six
         head pairs, computed via fp8-DR matmuls; interleaved between
         early phase-B tiles so the scalar engine's exps overlap
         tensor work.
  B:     v = attnT^T@Wv (ones col = softmax denominator), tT scores
         from attn8 via Weff_B (fp8-DR) -> exp -> xs accumulated
         directly in a persistent PSUM bank across all 32 seq tiles.
  AC-2:  x_out = pa^T @ xs_bd with ones-column denominators,
         normalize, PE-transpose, proj.

Head-major layout trick: the 2C projection outputs are permuted host-
side from (branch, head, d) to (head, branch, d) so each head pair
occupies one 128-partition tile; branch score scales (wa/wb * D^-0.5)
are folded into the K-side weights, so both branches' score maps come
out of a single contraction per head pair.

Bias handling: k-side biases that are constant along a softmax axis
cancel exactly and are dropped (bk_hf entirely; q-side bias of branch
A survives as the per-agent term c_A = k12bd^T @ bq, applied as the
exp's per-partition bias together with ba).
"""

import os
import sys
import numpy as np

for _p in ("/opt/trn_rl_repo", os.path.expanduser("~/.axon_site/_ro/trn_rl_repo")):
    if os.path.isdir(_p) and _p not in sys.path:
        sys.path.insert(0, _p)

import ml_dtypes

import concourse.bass as bass
import concourse.bacc as bacc
import concourse.tile as tile
from concourse import mybir
from concourse.bass_utils import run_bass_kernel_spmd

BF16 = mybir.dt.bfloat16
FP8 = mybir.dt.float8e4
F32 = mybir.dt.float32
NPBF16 = ml_dtypes.bfloat16
NPFP8 = ml_dtypes.float8_e4m3
DR = mybir.MatmulPerfMode.DoubleRow

B, N, NA, H, D = 8, 4096, 64, 12, 32
C = H * D            # 384
C2 = 2 * C           # 768
NP = H // 2          # 6 head pairs
NT = N // 128        # 32 seq tiles
SCALE = D ** -0.5
WSC = 64.0           # fp8 weight scale for the score contractions
ISC = 1.0 / WSC

_CACHE = {}


def _prune_ldweights(nc):
    """Drop InstLdweights whose weights AP matches the currently loaded one.

    The PE array keeps its stationary weights across matmuls; bass emits a
    fresh LDWEIGHTS per matmul regardless. Back-to-back matmuls that share
    lhsT (same AP, perf mode, tile geometry) only need the first load.
    Only bare loads (no semaphore waits/updates) are removed.
    """
    removed = 0
    for f in nc.m.functions:
        for bb in f.blocks:
            cur = None
            to_remove = []
            for inst in bb.instructions:
                tn = type(inst).__name__
                if getattr(inst, "engine", None) != mybir.EngineType.PE:
                    continue
                if tn == "InstLdweights":
                    ap = inst.ins[0]
                    sig = (ap.memref, ap.offset,
                           tuple(tuple(x) for x in ap.ap), str(ap.dtype),
                           str(getattr(inst, "perf_mode", None)),
                           bool(getattr(inst, "is_transpose", False)),
                           tuple(inst.tile_position) if inst.tile_position else None,
                           tuple(inst.tile_size) if inst.tile_size else None)
                    si = inst.sync_info
                    clean = si is None or (len(si.on_wait) == 0
                                           and len(si.on_update) == 0)
                    if cur == sig and clean:
                        to_remove.append(inst)
                    else:
                        cur = sig
                elif tn == "InstMatmult":
                    if getattr(inst, "is_transpose", False):
                        cur = None
                elif tn == "InstEventSemaphore":
                    pass
                else:
                    cur = None
            for inst in to_remove:
                bb.instructions.remove(inst)
                removed += 1
    return removed


def _build_bass(finalize=True, zero_bias=False, debug=False):
    nc = bacc.Bacc()

    # ---- DRAM I/O ----
    x8 = nc.dram_tensor("x8", [C, N], FP8, kind="ExternalInput")
    attn8 = nc.dram_tensor("attn8", [C, N], FP8, kind="ExternalInput")
    attnT = nc.dram_tensor("attnT", [C, N], BF16, kind="ExternalInput")
    agT = nc.dram_tensor("agT", [C, NA], BF16, kind="ExternalInput")
    wqT = nc.dram_tensor("wqT", [C2, C], BF16, kind="ExternalInput")
    wkag = nc.dram_tensor("wkag", [C, C2], BF16, kind="ExternalInput")
    wqag = nc.dram_tensor("wqag", [C, C2], BF16, kind="ExternalInput")
    wkhfT = nc.dram_tensor("wkhfT", [C2, C], BF16, kind="ExternalInput")
    wv = nc.dram_tensor("wv", [C, H * 33], BF16, kind="ExternalInput")
    wproj = nc.dram_tensor("wproj", [C, C], BF16, kind="ExternalInput")
    bq = nc.dram_tensor("bq", [C2], F32, kind="ExternalInput")
    bkag = nc.dram_tensor("bkag", [C2], F32, kind="ExternalInput")
    bqag = nc.dram_tensor("bqag", [C2], F32, kind="ExternalInput")
    bv = nc.dram_tensor("bv", [H * 33], F32, kind="ExternalInput")
    bproj = nc.dram_tensor("bproj", [C], F32, kind="ExternalInput")
    bab = nc.dram_tensor("bab", [2], F32, kind="ExternalInput")
    out = nc.dram_tensor("out", [N, C], F32, kind="ExternalOutput")
    if debug:
        d_pa = nc.dram_tensor("d_pa", [128, 6, N], BF16, kind="ExternalOutput")
        d_xsbd = nc.dram_tensor("d_xsbd", [128, 6 * 66], BF16, kind="ExternalOutput")
        d_pt0 = nc.dram_tensor("d_pt0", [128, C2], BF16, kind="ExternalOutput")
        d_v0 = nc.dram_tensor("d_v0", [128, H * 33], BF16, kind="ExternalOutput")
        d_weffa = nc.dram_tensor("d_weffa", [128, 4, C2], FP8, kind="ExternalOutput")
        d_weffb = nc.dram_tensor("d_weffb", [128, 4, C2], FP8, kind="ExternalOutput")
        d_pt5 = nc.dram_tensor("d_pt5", [128, C2], BF16, kind="ExternalOutput")
        d_v5 = nc.dram_tensor("d_v5", [128, H * 33], BF16, kind="ExternalOutput")
        d_xsraw = nc.dram_tensor("d_xsraw", [128, 6 * 66], F32, kind="ExternalOutput")

    Exp = mybir.ActivationFunctionType.Exp
    Copy = mybir.ActivationFunctionType.Copy

    with tile.TileContext(nc) as tc:
        with (
            tc.tile_pool(name="const", bufs=1) as const,
            tc.tile_pool(name="pt", bufs=2) as p_pt,
            tc.tile_pool(name="vv", bufs=2) as p_v,
            tc.tile_pool(name="xon", bufs=3) as p_xon,
            tc.tile_pool(name="xot", bufs=3) as p_xot,
            tc.tile_pool(name="osb", bufs=3) as p_out,
            tc.tile_pool(name="sm", bufs=4) as p_sm,
            tc.tile_pool(name="psA", bufs=2, space="PSUM") as psA,
            tc.tile_pool(name="psS", bufs=2, space="PSUM") as psS,
            tc.tile_pool(name="psV", bufs=1, space="PSUM") as psV,
            tc.tile_pool(name="psX", bufs=1, space="PSUM") as psX,
        ):
            # ---- small constants (gpsimd DMA queue); wproj last (AC-2 only)
            ag_t = const.tile([128, 3, NA], BF16)
            nc.gpsimd.dma_start(out=ag_t, in_=agT.rearrange("(k p) m -> p k m", p=128))
            w_kag = const.tile([128, 3, C2], BF16)
            w_qag = const.tile([128, 3, C2], BF16)
            w_qT = const.tile([128, 6, C], BF16)
            w_khfT = const.tile([128, 6, C], BF16)
            w_v = const.tile([128, 3, H * 33], BF16)
            w_pr = const.tile([128, 3, C], BF16)
            for dst, src in ((w_kag, wkag), (w_qag, wqag), (w_qT, wqT),
                             (w_khfT, wkhfT), (w_v, wv)):
                nc.gpsimd.dma_start(out=dst, in_=src.rearrange("(k p) m -> p k m", p=128))
            b_q = const.tile([128, 6], F32)
            b_kag = const.tile([128, 6], F32)
            b_qag = const.tile([128, 6], F32)
            for dst, src in ((b_q, bq), (b_kag, bkag), (b_qag, bqag)):
                nc.gpsimd.dma_start(out=dst, in_=src.rearrange("(j p) -> p j", p=128))
            bv_row = const.tile([1, H * 33], BF16)
            nc.gpsimd.dma_start(out=bv_row, in_=bv[:].unsqueeze(0))
            bpr_row = const.tile([1, C], BF16)
            nc.gpsimd.dma_start(out=bpr_row, in_=bproj[:].unsqueeze(0))
            ones_row = const.tile([1, 128], BF16)
            nc.vector.memset(ones_row, 1.0)
            ba_t = const.tile([128, 1], F32)
            nc.gpsimd.dma_start(out=ba_t, in_=bass.AP(tensor=bab[:].tensor, offset=0,
                                                      ap=[[0, 128], [1, 1]]))
            bb_t = const.tile([128, 1], F32)
            nc.gpsimd.dma_start(out=bb_t, in_=bass.AP(tensor=bab[:].tensor, offset=1,
                                                      ap=[[0, 128], [1, 1]]))
            nc.gpsimd.dma_start(out=w_pr, in_=wproj.rearrange("(k p) m -> p k m", p=128))

            # ---- big input streams, split across sync + scalar DMA queues,
            # small leading pieces so the first tiles unblock early ----
            x8_t = const.tile([128, 4, N], FP8)
            at8_t = const.tile([128, 4, N], FP8)
            att_t = const.tile([128, 3, N], BF16)
            nc.vector.memset(x8_t[:, 3, :], 0.0)
            nc.vector.memset(at8_t[:, 3, :], 0.0)
            x8r = x8.rearrange("(k p) s -> p k s", p=128)
            at8r = attn8.rearrange("(k p) s -> p k s", p=128)
            attr = attnT.rearrange("(k p) s -> p k s", p=128)
            bounds = (0, 256, 512, 1024, 2048, 3072, 4096)
            for lo, hi in zip(bounds[:-1], bounds[1:]):
                sl = slice(lo, hi)
                nc.sync.dma_start(out=at8_t[:, 0:3, sl], in_=at8r[:, :, sl])
                nc.scalar.dma_start(out=att_t[:, :, sl], in_=attr[:, :, sl])
                nc.sync.dma_start(out=x8_t[:, 0:3, sl], in_=x8r[:, :, sl])

            # Pre-touch small DMA constants so wide consumers only carry
            # the producing engine's wait.
            touch = const.tile([128, 16], F32)
            for i, t_ap in enumerate((b_q[:, 0:1], b_kag[:, 0:1], b_qag[:, 0:1],
                                      ba_t[:, 0:1], bb_t[:, 0:1])):
                nc.vector.tensor_copy(touch[:, i:i + 1], t_ap)
            nc.scalar.copy(touch[:, 8:9], ba_t[:, 0:1])
            nc.scalar.copy(touch[:, 9:10], bb_t[:, 0:1])

            # ---- prep: k_ag / qa projections -> block-diag tiles ----
            kag_sb = const.tile([128, 6, NA], BF16)
            qa_sb = const.tile([128, 6, NA], BF16)
            for w_t, b_t, dst in ((w_kag, b_kag, kag_sb), (w_qag, b_qag, qa_sb)):
                for j in range(6):
                    ps = psA.tile([128, NA], F32, tag="a")
                    for k in range(3):
                        nc.tensor.matmul(ps, lhsT=w_t[:, k, j * 128:(j + 1) * 128],
                                         rhs=ag_t[:, k, :], start=(k == 0), stop=(k == 2))
                    nc.vector.tensor_add(dst[:, j, :], ps,
                                         b_t[:, j:j + 1].to_broadcast([128, NA]))
            k12bd = const.tile([128, 6, 128], BF16)
            qabd = const.tile([128, 6, 128], BF16)
            for src, dst in ((kag_sb, k12bd), (qa_sb, qabd)):
                nc.vector.memset(dst, 0.0)
                for j in range(6):
                    nc.vector.tensor_copy(dst[0:64, j, 0:64], src[0:64, j, :])
                    nc.vector.tensor_copy(dst[64:128, j, 64:128], src[64:128, j, :])

            # ---- prep: effective score weights (fp8, scaled by WSC) ----
            weff_a8 = const.tile([128, 4, C2], FP8)
            weff_b8 = const.tile([128, 4, C2], FP8)
            nc.vector.memset(weff_a8[:, 3, :], 0.0)
            nc.vector.memset(weff_b8[:, 3, :], 0.0)
            for j in range(6):
                for k in range(3):
                    ps = psA.tile([128, 128], F32, tag="a")
                    nc.tensor.matmul(ps, lhsT=w_qT[:, j, k * 128:(k + 1) * 128],
                                     rhs=k12bd[:, j, :], start=True, stop=True)
                    nc.scalar.activation(weff_a8[:, k, j * 128:(j + 1) * 128], ps,
                                         Copy, scale=WSC)
                    ps2 = psA.tile([128, 128], F32, tag="a")
                    nc.tensor.matmul(ps2, lhsT=w_khfT[:, j, k * 128:(k + 1) * 128],
                                     rhs=qabd[:, j, :], start=True, stop=True)
                    nc.scalar.activation(weff_b8[:, k, j * 128:(j + 1) * 128], ps2,
                                         Copy, scale=WSC)
            cba = None
            if not zero_bias:
                b_q_bf = const.tile([128, 6], BF16)
                nc.vector.tensor_copy(b_q_bf, b_q)
                cba = const.tile([128, 6], F32)
                for j in range(6):
                    ps = psA.tile([128, 1], F32, tag="a")
                    nc.tensor.matmul(ps, lhsT=k12bd[:, j, :], rhs=b_q_bf[:, j:j + 1],
                                     start=True, stop=True)
                    nc.vector.tensor_add(cba[:, j:j + 1], ps, ba_t[:, 0:1])

            # ---- interleaved AC-1 (branch-A score maps) + phase B ----
            pa_t = const.tile([128, 6, N], BF16)
            xs_ps = psX.tile([128, 6 * 66], F32, tag="x")

            def ac1_unit(u):
                # one head pair x two seq chunks; shared lhsT back-to-back so
                # the redundant LDWEIGHTS prune below can drop half the loads
                j, cp = u // 4, u % 4
                tiles = []
                for ci in (2 * cp, 2 * cp + 1):
                    csl = slice(ci * 512, (ci + 1) * 512)
                    ps = psA.tile([128, 512], F32, tag="a", name=f"ac1_{u}_{ci}")
                    tiles.append((ps, csl))
                for kp in (0, 1):
                    for ps, csl in tiles:
                        nc.tensor.matmul(ps,
                                         lhsT=weff_a8[:, 2 * kp:2 * kp + 2,
                                                      j * 128:(j + 1) * 128],
                                         rhs=x8_t[:, 2 * kp:2 * kp + 2, csl],
                                         start=(kp == 0), stop=(kp == 1),
                                         perf_mode=DR)
                abias = 0.0 if zero_bias else cba[:, j:j + 1]
                for ps, csl in tiles:
                    nc.scalar.activation(pa_t[:, j, csl], ps, Exp, bias=abias,
                                         scale=ISC)

            for t in range(NT):
                if t < 24:
                    ac1_unit(t)
                tsl = slice(t * 128, (t + 1) * 128)
                # branch-B scores for this seq tile (fp8 DoubleRow)
                ps_s = psS.tile([128, C2], F32, tag="s")
                for kp in (0, 1):
                    for r0, r1 in ((0, 512), (512, 768)):
                        nc.tensor.matmul(ps_s[:, r0:r1],
                                         lhsT=at8_t[:, 2 * kp:2 * kp + 2, tsl],
                                         rhs=weff_b8[:, 2 * kp:2 * kp + 2, r0:r1],
                                         start=(kp == 0), stop=(kp == 1),
                                         perf_mode=DR)
                pt = p_pt.tile([128, C2], BF16)
                bbias = 0.0 if zero_bias else bb_t[:, 0:1]
                nc.scalar.activation(pt, ps_s, Exp, bias=bbias, scale=ISC)
                # v for this seq tile
                psv = psV.tile([128, H * 33], F32, tag="v")
                for k in range(3):
                    nc.tensor.matmul(psv, lhsT=att_t[:, k, tsl], rhs=w_v[:, k, :],
                                     start=(k == 0), stop=zero_bias and (k == 2))
                v_t = p_v.tile([128, H * 33], BF16)
                v3 = v_t[:].rearrange("p (h c) -> p h c", c=33)
                if zero_bias:
                    pv3 = psv.rearrange("p (h c) -> p h c", c=33)
                    nc.vector.tensor_copy(v3[:, :, 0:32], pv3[:, :, 0:32])
                    nc.vector.memset(v3[:, :, 32], 1.0)
                else:
                    nc.tensor.matmul(psv, lhsT=ones_row[:, :], rhs=bv_row[:, :],
                                     start=False, stop=True)
                    nc.vector.tensor_copy(v_t, psv)
                # xs accumulation directly in PSUM across all tiles
                for j in range(6):
                    # start only on the very first write: start_tensor_calc
                    # marks the whole 2KB bank pending-zero, so each later
                    # region's first write overwrites, then accumulates.
                    nc.tensor.matmul(xs_ps[:, j * 66:(j + 1) * 66],
                                     lhsT=pt[:, j * 128:(j + 1) * 128],
                                     rhs=v_t[:, j * 66:(j + 1) * 66],
                                     start=(t == 0 and j == 0), stop=(t == NT - 1),
                                     skip_group_check=True)
                if debug and t == 0:
                    nc.sync.dma_start(out=d_pt0[:, :], in_=pt)
                    nc.sync.dma_start(out=d_v0[:, :], in_=v_t)
                if debug and t == 5:
                    nc.sync.dma_start(out=d_pt5[:, :], in_=pt)
                    nc.sync.dma_start(out=d_v5[:, :], in_=v_t)

            # ---- xs normalize -> block-diag [xs | 1] tiles ----
            xs_bd = const.tile([128, 6 * 66], BF16)
            xs3 = xs_ps.rearrange("p (j c) -> p j c", c=66)
            bd3 = xs_bd[:].rearrange("p (j c) -> p j c", c=66)
            nc.vector.memset(xs_bd, 0.0)
            nc.vector.memset(bd3[0:64, :, 32:33], 1.0)
            nc.vector.memset(bd3[64:128, :, 65:66], 1.0)
            rec6 = p_sm.tile([128, 6], F32, tag="rec")
            nc.vector.reciprocal(rec6[0:64, :], xs3[0:64, :, 32])
            nc.vector.reciprocal(rec6[64:128, :], xs3[64:128, :, 65])
            nc.vector.tensor_mul(bd3[0:64, :, 0:32], xs3[0:64, :, 0:32],
                                 rec6[0:64, :].unsqueeze(2).to_broadcast([64, 6, 32]))
            nc.vector.tensor_mul(bd3[64:128, :, 33:65], xs3[64:128, :, 33:65],
                                 rec6[64:128, :].unsqueeze(2).to_broadcast([64, 6, 32]))

            if debug:
                xsr_sb = const.tile([128, 6 * 66], F32)
                nc.scalar.copy(xsr_sb, xs_ps)
                nc.sync.dma_start(out=d_xsraw[:, :], in_=xsr_sb)
                nc.sync.dma_start(out=d_pa[:, :, :], in_=pa_t)
                nc.sync.dma_start(out=d_xsbd[:, :], in_=xs_bd)
                nc.sync.dma_start(out=d_weffa[:, :, :], in_=weff_a8)
                nc.sync.dma_start(out=d_weffb[:, :, :], in_=weff_b8)

            # ---- AC-2: x_out, normalize, transpose, proj ----
            for t in range(NT):
                tsl = slice(t * 128, (t + 1) * 128)
                xo_ps = psA.tile([128, 12 * 33], F32, tag="a", name=f"xo_{t}")
                for j in range(6):
                    nc.tensor.matmul(xo_ps[:, j * 66:(j + 1) * 66],
                                     lhsT=pa_t[:, j, tsl],
                                     rhs=xs_bd[:, j * 66:(j + 1) * 66],
                                     start=True, stop=True)
                xo3 = xo_ps.rearrange("p (k c) -> p k c", c=33)
                rec = p_sm.tile([128, 12], F32, tag="rec12")
                nc.vector.reciprocal(rec, xo3[:, :, 32])
                xon = p_xon.tile([128, C], BF16)
                nc.vector.tensor_mul(xon[:].rearrange("p (k c) -> p k c", c=32),
                                     xo3[:, :, 0:32],
                                     rec[:].unsqueeze(2).to_broadcast([128, 12, 32]))
                xot = p_xot.tile([128, 3, 128], BF16)
                nc.scalar.dma_start_transpose(out=xot, in_=xon)
                pr_ps = psS.tile([128, C], F32, tag="s", name=f"pr_{t}")
                for f in range(3):
                    nc.tensor.matmul(pr_ps, lhsT=xot[:, f, :],
                                     rhs=w_pr[:, f, :],
                                     start=(f == 0),
                                     stop=(zero_bias and f == 2),
                                     skip_group_check=True)
                if not zero_bias:
                    nc.tensor.matmul(pr_ps, lhsT=ones_row[:, :], rhs=bpr_row[:, :],
                                     start=False, stop=True, skip_group_check=True)
                o_sb = p_out.tile([128, C], F32)
                nc.vector.tensor_copy(o_sb, pr_ps)
                nc.gpsimd.dma_start(out=out[t * 128:(t + 1) * 128, :], in_=o_sb)
    _prune_ldweights(nc)
    if finalize:
        nc.finalize()
    return nc


def _prep_host(inputs):
    f32 = np.float32
    x = np.asarray(inputs["x"], f32)
    attn = np.asarray(inputs["attn"], f32)
    agent = np.asarray(inputs["agent_input"], f32)
    wa = np.asarray(inputs["wa"], f32)
    wb = np.asarray(inputs["wb"], f32)

    perm = np.empty(C2, np.int64)
    sva = np.empty(C2, f32)
    svb = np.empty(C2, f32)
    for h in range(H):
        for br in range(2):
            j0 = h * 64 + br * 32
            perm[j0:j0 + 32] = br * C + h * 32 + np.arange(32)
            sva[j0:j0 + 32] = wa[br] * SCALE
            svb[j0:j0 + 32] = wb[br] * SCALE

    wq_p = np.asarray(inputs["Wq_lf"], f32)[:, perm]
    bq_p = np.asarray(inputs["bq_lf"], f32)[perm]
    wkag_p = np.asarray(inputs["Wk_ag"], f32)[:, perm] * sva[None, :]
    bkag_p = np.asarray(inputs["bk_ag"], f32)[perm] * sva
    wqag_p = np.asarray(inputs["Wq_ag"], f32)[:, perm]
    bqag_p = np.asarray(inputs["bq_ag"], f32)[perm]
    wkhf_p = np.asarray(inputs["Wk_hf"], f32)[:, perm] * svb[None, :]

    wv_in = np.asarray(inputs["Wv_hf"], f32)
    bv_in = np.asarray(inputs["bv_hf"], f32)
    wv_aug = np.zeros((C, H * 33), f32)
    bv_aug = np.zeros(H * 33, f32)
    for h in range(H):
        wv_aug[:, h * 33:h * 33 + 32] = wv_in[:, h * 32:h * 32 + 32]
        bv_aug[h * 33:h * 33 + 32] = bv_in[h * 32:h * 32 + 32]
        bv_aug[h * 33 + 32] = 1.0

    bab = np.array([np.asarray(inputs["ba"], f32)[0],
                    np.asarray(inputs["bb"], f32)[0]], f32)

    shared = {
        "wqT": np.ascontiguousarray(wq_p.T).astype(NPBF16),
        "wkhfT": np.ascontiguousarray(wkhf_p.T).astype(NPBF16),
        "wkag": wkag_p.astype(NPBF16),
        "wqag": wqag_p.astype(NPBF16),
        "wv": wv_aug.astype(NPBF16),
        "wproj": np.asarray(inputs["Wproj"], f32).astype(NPBF16),
        "bq": bq_p, "bkag": bkag_p, "bqag": bqag_p,
        "bv": bv_aug, "bproj": np.ascontiguousarray(np.asarray(inputs["bproj"], f32)),
        "bab": bab,
    }
    xT = np.ascontiguousarray(x.transpose(0, 2, 1))
    attnT = np.ascontiguousarray(attn.transpose(0, 2, 1))
    agT = np.ascontiguousarray(agent.transpose(0, 2, 1)).astype(NPBF16)
    x8 = xT.astype(NPFP8)
    attn8 = attnT.astype(NPFP8)
    attnTb = attnT.astype(NPBF16)
    in_maps = []
    for b in range(B):
        m = dict(shared)
        m["x8"] = x8[b]
        m["attn8"] = attn8[b]
        m["attnT"] = attnTb[b]
        m["agT"] = agT[b]
        in_maps.append(m)
    return in_maps


def kernel(**inputs):
    zb = all(not np.any(np.asarray(inputs[k]))
             for k in ("bq_lf", "bk_ag", "bq_ag", "bk_hf", "bv_hf", "bproj",
                       "ba", "bb"))
    key = ("nc", zb)
    if key not in _CACHE:
        _CACHE[key] = _build_bass(zero_bias=zb)
    nc = _CACHE[key]
    in_maps = _prep_host(inputs)
    res = run_bass_kernel_spmd(nc, in_maps, core_ids=list(range(B)))
    return np.stack([res.results[b]["out"] for b in range(B)], axis=0)


# revision 27
# speedup vs baseline: 1.5831x; 1.4351x over previous
"""Trainium2 Bass kernel for the dual-branch agent-attention module.

Sharding: data-parallel over B=8 (one batch element per NeuronCore).
All transposes and weight permutations are done host-side; on-device
work is a streamed pipeline with the two big score contractions run in
fp8e4m3 DoubleRow mode (4 rows of K per PE pass -> 3x fewer tensor
cycles than bf16), which is safe here because the effective score
weights are tiny (|s| < 0.7) and the softmax is near-uniform.

Structure:
  prep:  agent projections k_ag/qa -> block-diagonal tiles; effective
         score weights Weff_A = Wq @ k12bd and Weff_B = Wkhf @ qabd
         (the big activations never materialize q or kh), written as
         fp8 scaled by 64 (undone via the exp activation's scale).
  AC-1:  branch-A score maps pa = exp(x @ Weff_A / 64) for all six
         head pairs, computed via fp8-DR matmuls; interleaved between
         early phase-B tiles so the scalar engine's exps overlap
         tensor work.
  B:     v = attnT^T@Wv (ones col = softmax denominator), tT scores
         from attn8 via Weff_B (fp8-DR) -> exp -> xs accumulated
         directly in a persistent PSUM bank across all 32 seq tiles.
  AC-2:  x_out = pa^T @ xs_bd with ones-column denominators,
         normalize, PE-transpose, proj.

Head-major layout trick: the 2C projection outputs are permuted host-
side from (branch, head, d) to (head, branch, d) so each head pair
occupies one 128-partition tile; branch score scales (wa/wb * D^-0.5)
are folded into the K-side weights, so both branches' score maps come
out of a single contraction per head pair.

Bias handling: k-side biases that are constant along a softmax axis
cancel exactly and are dropped (bk_hf entirely; q-side bias of branch
A survives as the per-agent term c_A = k12bd^T @ bq, applied as the
exp's per-partition bias together with ba).
"""

import os
import sys
import numpy as np

for _p in ("/opt/trn_rl_repo", os.path.expanduser("~/.axon_site/_ro/trn_rl_repo")):
    if os.path.isdir(_p) and _p not in sys.path:
        sys.path.insert(0, _p)

import ml_dtypes

import concourse.bass as bass
import concourse.bacc as bacc
import concourse.tile as tile
from concourse import mybir
from concourse.bass_utils import run_bass_kernel_spmd
from concourse.masks import make_identity

BF16 = mybir.dt.bfloat16
FP8 = mybir.dt.float8e4
F32 = mybir.dt.float32
NPBF16 = ml_dtypes.bfloat16
NPFP8 = ml_dtypes.float8_e4m3
DR = mybir.MatmulPerfMode.DoubleRow

B, N, NA, H, D = 8, 4096, 64, 12, 32
C = H * D            # 384
C2 = 2 * C           # 768
NP = H // 2          # 6 head pairs
NT = N // 128        # 32 seq tiles
SCALE = D ** -0.5
WSC = 64.0           # fp8 weight scale for the score contractions
ISC = 1.0 / WSC

_CACHE = {}


def _prune_ldweights(nc):
    """Drop InstLdweights whose weights AP matches the currently loaded one.

    The PE array keeps its stationary weights across matmuls; bass emits a
    fresh LDWEIGHTS per matmul regardless. Back-to-back matmuls that share
    lhsT (same AP, perf mode, tile geometry) only need the first load.
    Only bare loads (no semaphore waits/updates) are removed.
    """
    removed = 0
    for f in nc.m.functions:
        for bb in f.blocks:
            cur = None
            to_remove = []
            for inst in bb.instructions:
                tn = type(inst).__name__
                if getattr(inst, "engine", None) != mybir.EngineType.PE:
                    continue
                if tn == "InstLdweights":
                    ap = inst.ins[0]
                    sig = (ap.memref, ap.offset,
                           tuple(tuple(x) for x in ap.ap), str(ap.dtype),
                           str(getattr(inst, "perf_mode", None)),
                           bool(getattr(inst, "is_transpose", False)),
                           tuple(inst.tile_position) if inst.tile_position else None,
                           tuple(inst.tile_size) if inst.tile_size else None)
                    si = inst.sync_info
                    clean = si is None or (len(si.on_wait) == 0
                                           and len(si.on_update) == 0)
                    if cur == sig and clean:
                        to_remove.append(inst)
                    else:
                        cur = sig
                elif tn == "InstMatmult":
                    if getattr(inst, "is_transpose", False):
                        cur = None
                elif tn == "InstEventSemaphore":
                    pass
                else:
                    cur = None
            for inst in to_remove:
                bb.instructions.remove(inst)
                removed += 1
    return removed


def _build_bass(finalize=True, zero_bias=False, debug=False):
    nc = bacc.Bacc()

    # ---- DRAM I/O ----
    x8 = nc.dram_tensor("x8", [C, N], FP8, kind="ExternalInput")
    attn8 = nc.dram_tensor("attn8", [C, N], FP8, kind="ExternalInput")
    attnT = nc.dram_tensor("attnT", [C, N], BF16, kind="ExternalInput")
    agT = nc.dram_tensor("agT", [C, NA], BF16, kind="ExternalInput")
    wqT = nc.dram_tensor("wqT", [C2, C], BF16, kind="ExternalInput")
    wkag = nc.dram_tensor("wkag", [C, C2], BF16, kind="ExternalInput")
    wqag = nc.dram_tensor("wqag", [C, C2], BF16, kind="ExternalInput")
    wkhfT = nc.dram_tensor("wkhfT", [C2, C], BF16, kind="ExternalInput")
    wv = nc.dram_tensor("wv", [C, H * 33], BF16, kind="ExternalInput")
    wproj = nc.dram_tensor("wproj", [C, C], BF16, kind="ExternalInput")
    bq = nc.dram_tensor("bq", [C2], F32, kind="ExternalInput")
    bkag = nc.dram_tensor("bkag", [C2], F32, kind="ExternalInput")
    bqag = nc.dram_tensor("bqag", [C2], F32, kind="ExternalInput")
    bv = nc.dram_tensor("bv", [H * 33], F32, kind="ExternalInput")
    bproj = nc.dram_tensor("bproj", [C], F32, kind="ExternalInput")
    bab = nc.dram_tensor("bab", [2], F32, kind="ExternalInput")
    out = nc.dram_tensor("out", [N, C], F32, kind="ExternalOutput")
    if debug:
        d_pa = nc.dram_tensor("d_pa", [128, 6, N], BF16, kind="ExternalOutput")
        d_xsbd = nc.dram_tensor("d_xsbd", [128, 6 * 66], BF16, kind="ExternalOutput")
        d_pt0 = nc.dram_tensor("d_pt0", [128, C2], BF16, kind="ExternalOutput")
        d_v0 = nc.dram_tensor("d_v0", [128, H * 33], BF16, kind="ExternalOutput")
        d_weffa = nc.dram_tensor("d_weffa", [128, 4, C2], FP8, kind="ExternalOutput")
        d_weffb = nc.dram_tensor("d_weffb", [128, 4, C2], FP8, kind="ExternalOutput")
        d_pt5 = nc.dram_tensor("d_pt5", [128, C2], BF16, kind="ExternalOutput")
        d_v5 = nc.dram_tensor("d_v5", [128, H * 33], BF16, kind="ExternalOutput")
        d_xsraw = nc.dram_tensor("d_xsraw", [128, 6 * 66], F32, kind="ExternalOutput")

    Exp = mybir.ActivationFunctionType.Exp
    Copy = mybir.ActivationFunctionType.Copy

    with tile.TileContext(nc) as tc:
        with (
            tc.tile_pool(name="const", bufs=1) as const,
            tc.tile_pool(name="pt", bufs=2) as p_pt,
            tc.tile_pool(name="vv", bufs=2) as p_v,
            tc.tile_pool(name="xon", bufs=3) as p_xon,
            tc.tile_pool(name="xot", bufs=3) as p_xot,
            tc.tile_pool(name="osb", bufs=3) as p_out,
            tc.tile_pool(name="sm", bufs=4) as p_sm,
            tc.tile_pool(name="psA", bufs=2, space="PSUM") as psA,
            tc.tile_pool(name="psS", bufs=2, space="PSUM") as psS,
            tc.tile_pool(name="psV", bufs=1, space="PSUM") as psV,
            tc.tile_pool(name="psX", bufs=1, space="PSUM") as psX,
        ):
            # ---- small constants (gpsimd DMA queue); wproj last (AC-2 only)
            ag_t = const.tile([128, 3, NA], BF16)
            nc.gpsimd.dma_start(out=ag_t, in_=agT.rearrange("(k p) m -> p k m", p=128))
            w_kag = const.tile([128, 3, C2], BF16)
            w_qag = const.tile([128, 3, C2], BF16)
            w_qT = const.tile([128, 6, C], BF16)
            w_khfT = const.tile([128, 6, C], BF16)
            w_v = const.tile([128, 3, H * 33], BF16)
            w_pr = const.tile([128, 3, C], BF16)
            for dst, src in ((w_kag, wkag), (w_qag, wqag), (w_qT, wqT),
                             (w_khfT, wkhfT), (w_v, wv)):
                nc.gpsimd.dma_start(out=dst, in_=src.rearrange("(k p) m -> p k m", p=128))
            b_q = const.tile([128, 6], F32)
            b_kag = const.tile([128, 6], F32)
            b_qag = const.tile([128, 6], F32)
            for dst, src in ((b_q, bq), (b_kag, bkag), (b_qag, bqag)):
                nc.gpsimd.dma_start(out=dst, in_=src.rearrange("(j p) -> p j", p=128))
            bv_row = const.tile([1, H * 33], BF16)
            nc.gpsimd.dma_start(out=bv_row, in_=bv[:].unsqueeze(0))
            bpr_row = const.tile([1, C], BF16)
            nc.gpsimd.dma_start(out=bpr_row, in_=bproj[:].unsqueeze(0))
            ones_row = const.tile([1, 128], BF16)
            nc.vector.memset(ones_row, 1.0)
            ba_t = const.tile([128, 1], F32)
            nc.gpsimd.dma_start(out=ba_t, in_=bass.AP(tensor=bab[:].tensor, offset=0,
                                                      ap=[[0, 128], [1, 1]]))
            bb_t = const.tile([128, 1], F32)
            nc.gpsimd.dma_start(out=bb_t, in_=bass.AP(tensor=bab[:].tensor, offset=1,
                                                      ap=[[0, 128], [1, 1]]))
            nc.gpsimd.dma_start(out=w_pr, in_=wproj.rearrange("(k p) m -> p k m", p=128))
            ident = const.tile([128, 128], BF16)
            make_identity(nc, ident)

            # ---- big input streams, split across sync + scalar DMA queues,
            # small leading pieces so the first tiles unblock early ----
            x8_t = const.tile([128, 4, N], FP8)
            at8_t = const.tile([128, 4, N], FP8)
            att_t = const.tile([128, 3, N], BF16)
            nc.vector.memset(x8_t[:, 3, :], 0.0)
            nc.vector.memset(at8_t[:, 3, :], 0.0)
            x8r = x8.rearrange("(k p) s -> p k s", p=128)
            at8r = attn8.rearrange("(k p) s -> p k s", p=128)
            attr = attnT.rearrange("(k p) s -> p k s", p=128)
            bounds = (0, 256, 512, 1024, 2048, 3072, 4096)
            for lo, hi in zip(bounds[:-1], bounds[1:]):
                sl = slice(lo, hi)
                nc.sync.dma_start(out=at8_t[:, 0:3, sl], in_=at8r[:, :, sl])
                nc.scalar.dma_start(out=att_t[:, :, sl], in_=attr[:, :, sl])
                nc.sync.dma_start(out=x8_t[:, 0:3, sl], in_=x8r[:, :, sl])

            # Pre-touch small DMA constants so wide consumers only carry
            # the producing engine's wait.
            touch = const.tile([128, 16], F32)
            for i, t_ap in enumerate((b_q[:, 0:1], b_kag[:, 0:1], b_qag[:, 0:1],
                                      ba_t[:, 0:1], bb_t[:, 0:1])):
                nc.vector.tensor_copy(touch[:, i:i + 1], t_ap)
            nc.scalar.copy(touch[:, 8:9], ba_t[:, 0:1])
            nc.scalar.copy(touch[:, 9:10], bb_t[:, 0:1])

            # ---- prep: k_ag / qa projections -> block-diag tiles ----
            kag_sb = const.tile([128, 6, NA], BF16)
            qa_sb = const.tile([128, 6, NA], BF16)
            for w_t, b_t, dst in ((w_kag, b_kag, kag_sb), (w_qag, b_qag, qa_sb)):
                for j in range(6):
                    ps = psA.tile([128, NA], F32, tag="a")
                    for k in range(3):
                        nc.tensor.matmul(ps, lhsT=w_t[:, k, j * 128:(j + 1) * 128],
                                         rhs=ag_t[:, k, :], start=(k == 0), stop=(k == 2))
                    nc.vector.tensor_add(dst[:, j, :], ps,
                                         b_t[:, j:j + 1].to_broadcast([128, NA]))
            k12bd = const.tile([128, 6, 128], BF16)
            qabd = const.tile([128, 6, 128], BF16)
            for src, dst in ((kag_sb, k12bd), (qa_sb, qabd)):
                nc.vector.memset(dst, 0.0)
                for j in range(6):
                    nc.vector.tensor_copy(dst[0:64, j, 0:64], src[0:64, j, :])
                    nc.vector.tensor_copy(dst[64:128, j, 64:128], src[64:128, j, :])

            # ---- prep: effective score weights (fp8, scaled by WSC) ----
            weff_a8 = const.tile([128, 4, C2], FP8)
            weff_b8 = const.tile([128, 4, C2], FP8)
            nc.vector.memset(weff_a8[:, 3, :], 0.0)
            nc.vector.memset(weff_b8[:, 3, :], 0.0)
            for j in range(6):
                for k in range(3):
                    ps = psA.tile([128, 128], F32, tag="a")
                    nc.tensor.matmul(ps, lhsT=w_qT[:, j, k * 128:(k + 1) * 128],
                                     rhs=k12bd[:, j, :], start=True, stop=True)
                    nc.scalar.activation(weff_a8[:, k, j * 128:(j + 1) * 128], ps,
                                         Copy, scale=WSC)
                    ps2 = psA.tile([128, 128], F32, tag="a")
                    nc.tensor.matmul(ps2, lhsT=w_khfT[:, j, k * 128:(k + 1) * 128],
                                     rhs=qabd[:, j, :], start=True, stop=True)
                    nc.scalar.activation(weff_b8[:, k, j * 128:(j + 1) * 128], ps2,
                                         Copy, scale=WSC)
            cba = None
            if not zero_bias:
                b_q_bf = const.tile([128, 6], BF16)
                nc.vector.tensor_copy(b_q_bf, b_q)
                cba = const.tile([128, 6], F32)
                for j in range(6):
                    ps = psA.tile([128, 1], F32, tag="a")
                    nc.tensor.matmul(ps, lhsT=k12bd[:, j, :], rhs=b_q_bf[:, j:j + 1],
                                     start=True, stop=True)
                    nc.vector.tensor_add(cba[:, j:j + 1], ps, ba_t[:, 0:1])

            # ---- interleaved AC-1 (branch-A score maps) + phase B ----
            pa_t = const.tile([128, 6, N], BF16)
            xs_ps = psX.tile([128, 6 * 66], F32, tag="x")

            def ac1_unit(u):
                # one head pair x two seq chunks; shared lhsT back-to-back so
                # the redundant LDWEIGHTS prune below can drop half the loads
                j, cp = u // 4, u % 4
                tiles = []
                for ci in (2 * cp, 2 * cp + 1):
                    csl = slice(ci * 512, (ci + 1) * 512)
                    ps = psA.tile([128, 512], F32, tag="a", name=f"ac1_{u}_{ci}")
                    tiles.append((ps, csl))
                for kp in (0, 1):
                    for ps, csl in tiles:
                        nc.tensor.matmul(ps,
                                         lhsT=weff_a8[:, 2 * kp:2 * kp + 2,
                                                      j * 128:(j + 1) * 128],
                                         rhs=x8_t[:, 2 * kp:2 * kp + 2, csl],
                                         start=(kp == 0), stop=(kp == 1),
                                         perf_mode=DR)
                abias = 0.0 if zero_bias else cba[:, j:j + 1]
                for ps, csl in tiles:
                    nc.scalar.activation(pa_t[:, j, csl], ps, Exp, bias=abias,
                                         scale=ISC)

            for t in range(NT):
                if t < 24:
                    ac1_unit(t)
                tsl = slice(t * 128, (t + 1) * 128)
                # branch-B scores for this seq tile (fp8 DoubleRow)
                ps_s = psS.tile([128, C2], F32, tag="s")
                for kp in (0, 1):
                    for r0, r1 in ((0, 512), (512, 768)):
                        nc.tensor.matmul(ps_s[:, r0:r1],
                                         lhsT=at8_t[:, 2 * kp:2 * kp + 2, tsl],
                                         rhs=weff_b8[:, 2 * kp:2 * kp + 2, r0:r1],
                                         start=(kp == 0), stop=(kp == 1),
                                         perf_mode=DR)
                pt = p_pt.tile([128, C2], BF16)
                bbias = 0.0 if zero_bias else bb_t[:, 0:1]
                nc.scalar.activation(pt, ps_s, Exp, bias=bbias, scale=ISC)
                # v for this seq tile
                psv = psV.tile([128, H * 33], F32, tag="v")
                for k in range(3):
                    nc.tensor.matmul(psv, lhsT=att_t[:, k, tsl], rhs=w_v[:, k, :],
                                     start=(k == 0), stop=zero_bias and (k == 2))
                v_t = p_v.tile([128, H * 33], BF16)
                v3 = v_t[:].rearrange("p (h c) -> p h c", c=33)
                if zero_bias:
                    pv3 = psv.rearrange("p (h c) -> p h c", c=33)
                    nc.vector.tensor_copy(v3[:, :, 0:32], pv3[:, :, 0:32])
                    nc.vector.memset(v3[:, :, 32], 1.0)
                else:
                    nc.tensor.matmul(psv, lhsT=ones_row[:, :], rhs=bv_row[:, :],
                                     start=False, stop=True)
                    nc.vector.tensor_copy(v_t, psv)
                # xs accumulation directly in PSUM across all tiles
                for j in range(6):
                    # start only on the very first write: start_tensor_calc
                    # marks the whole 2KB bank pending-zero, so each later
                    # region's first write overwrites, then accumulates.
                    nc.tensor.matmul(xs_ps[:, j * 66:(j + 1) * 66],
                                     lhsT=pt[:, j * 128:(j + 1) * 128],
                                     rhs=v_t[:, j * 66:(j + 1) * 66],
                                     start=(t == 0 and j == 0), stop=(t == NT - 1),
                                     skip_group_check=True)
                if debug and t == 0:
                    nc.sync.dma_start(out=d_pt0[:, :], in_=pt)
                    nc.sync.dma_start(out=d_v0[:, :], in_=v_t)
                if debug and t == 5:
                    nc.sync.dma_start(out=d_pt5[:, :], in_=pt)
                    nc.sync.dma_start(out=d_v5[:, :], in_=v_t)

            # ---- xs normalize -> block-diag [xs | 1] tiles ----
            xs_bd = const.tile([128, 6 * 66], BF16)
            xs3 = xs_ps.rearrange("p (j c) -> p j c", c=66)
            bd3 = xs_bd[:].rearrange("p (j c) -> p j c", c=66)
            nc.vector.memset(xs_bd, 0.0)
            nc.vector.memset(bd3[0:64, :, 32:33], 1.0)
            nc.vector.memset(bd3[64:128, :, 65:66], 1.0)
            rec6 = p_sm.tile([128, 6], F32, tag="rec")
            nc.vector.reciprocal(rec6[0:64, :], xs3[0:64, :, 32])
            nc.vector.reciprocal(rec6[64:128, :], xs3[64:128, :, 65])
            nc.vector.tensor_mul(bd3[0:64, :, 0:32], xs3[0:64, :, 0:32],
                                 rec6[0:64, :].unsqueeze(2).to_broadcast([64, 6, 32]))
            nc.vector.tensor_mul(bd3[64:128, :, 33:65], xs3[64:128, :, 33:65],
                                 rec6[64:128, :].unsqueeze(2).to_broadcast([64, 6, 32]))

            if debug:
                xsr_sb = const.tile([128, 6 * 66], F32)
                nc.scalar.copy(xsr_sb, xs_ps)
                nc.sync.dma_start(out=d_xsraw[:, :], in_=xsr_sb)
                nc.sync.dma_start(out=d_pa[:, :, :], in_=pa_t)
                nc.sync.dma_start(out=d_xsbd[:, :], in_=xs_bd)
                nc.sync.dma_start(out=d_weffa[:, :, :], in_=weff_a8)
                nc.sync.dma_start(out=d_weffb[:, :, :], in_=weff_b8)

            # ---- AC-2: x_out, normalize, transpose, proj ----
            for t in range(NT):
                tsl = slice(t * 128, (t + 1) * 128)
                xo_ps = psA.tile([128, 12 * 33], F32, tag="a", name=f"xo_{t}")
                for j in range(6):
                    nc.tensor.matmul(xo_ps[:, j * 66:(j + 1) * 66],
                                     lhsT=pa_t[:, j, tsl],
                                     rhs=xs_bd[:, j * 66:(j + 1) * 66],
                                     start=True, stop=True)
                xo3 = xo_ps.rearrange("p (k c) -> p k c", c=33)
                rec = p_sm.tile([128, 12], F32, tag="rec12")
                nc.vector.reciprocal(rec, xo3[:, :, 32])
                xon = p_xon.tile([128, C], BF16)
                nc.vector.tensor_mul(xon[:].rearrange("p (k c) -> p k c", c=32),
                                     xo3[:, :, 0:32],
                                     rec[:].unsqueeze(2).to_broadcast([128, 12, 32]))
                tp_ps = psA.tile([128, C], BF16, tag="a", name=f"tp_{t}")
                for f in range(3):
                    nc.tensor.transpose(tp_ps[:, f * 128:(f + 1) * 128],
                                        xon[:, f * 128:(f + 1) * 128], ident)
                xot = p_xot.tile([128, C], BF16)
                nc.vector.tensor_copy(xot, tp_ps)
                pr_ps = psS.tile([128, C], F32, tag="s", name=f"pr_{t}")
                for f in range(3):
                    nc.tensor.matmul(pr_ps, lhsT=xot[:, f * 128:(f + 1) * 128],
                                     rhs=w_pr[:, f, :],
                                     start=(f == 0),
                                     stop=(zero_bias and f == 2),
                                     skip_group_check=True)
                if not zero_bias:
                    nc.tensor.matmul(pr_ps, lhsT=ones_row[:, :], rhs=bpr_row[:, :],
                                     start=False, stop=True, skip_group_check=True)
                o_sb = p_out.tile([128, C], F32)
                nc.scalar.copy(o_sb, pr_ps)
                nc.sync.dma_start(out=out[t * 128:(t + 1) * 128, :], in_=o_sb)
    _prune_ldweights(nc)
    if finalize:
        nc.finalize()
    return nc


def _prep_host(inputs):
    f32 = np.float32
    x = np.asarray(inputs["x"], f32)
    attn = np.asarray(inputs["attn"], f32)
    agent = np.asarray(inputs["agent_input"], f32)
    wa = np.asarray(inputs["wa"], f32)
    wb = np.asarray(inputs["wb"], f32)

    perm = np.empty(C2, np.int64)
    sva = np.empty(C2, f32)
    svb = np.empty(C2, f32)
    for h in range(H):
        for br in range(2):
            j0 = h * 64 + br * 32
            perm[j0:j0 + 32] = br * C + h * 32 + np.arange(32)
            sva[j0:j0 + 32] = wa[br] * SCALE
            svb[j0:j0 + 32] = wb[br] * SCALE

    wq_p = np.asarray(inputs["Wq_lf"], f32)[:, perm]
    bq_p = np.asarray(inputs["bq_lf"], f32)[perm]
    wkag_p = np.asarray(inputs["Wk_ag"], f32)[:, perm] * sva[None, :]
    bkag_p = np.asarray(inputs["bk_ag"], f32)[perm] * sva
    wqag_p = np.asarray(inputs["Wq_ag"], f32)[:, perm]
    bqag_p = np.asarray(inputs["bq_ag"], f32)[perm]
    wkhf_p = np.asarray(inputs["Wk_hf"], f32)[:, perm] * svb[None, :]

    wv_in = np.asarray(inputs["Wv_hf"], f32)
    bv_in = np.asarray(inputs["bv_hf"], f32)
    wv_aug = np.zeros((C, H * 33), f32)
    bv_aug = np.zeros(H * 33, f32)
    for h in range(H):
        wv_aug[:, h * 33:h * 33 + 32] = wv_in[:, h * 32:h * 32 + 32]
        bv_aug[h * 33:h * 33 + 32] = bv_in[h * 32:h * 32 + 32]
        bv_aug[h * 33 + 32] = 1.0

    bab = np.array([np.asarray(inputs["ba"], f32)[0],
                    np.asarray(inputs["bb"], f32)[0]], f32)

    shared = {
        "wqT": np.ascontiguousarray(wq_p.T).astype(NPBF16),
        "wkhfT": np.ascontiguousarray(wkhf_p.T).astype(NPBF16),
        "wkag": wkag_p.astype(NPBF16),
        "wqag": wqag_p.astype(NPBF16),
        "wv": wv_aug.astype(NPBF16),
        "wproj": np.asarray(inputs["Wproj"], f32).astype(NPBF16),
        "bq": bq_p, "bkag": bkag_p, "bqag": bqag_p,
        "bv": bv_aug, "bproj": np.ascontiguousarray(np.asarray(inputs["bproj"], f32)),
        "bab": bab,
    }
    xT = np.ascontiguousarray(x.transpose(0, 2, 1))
    attnT = np.ascontiguousarray(attn.transpose(0, 2, 1))
    agT = np.ascontiguousarray(agent.transpose(0, 2, 1)).astype(NPBF16)
    x8 = xT.astype(NPFP8)
    attn8 = attnT.astype(NPFP8)
    attnTb = attnT.astype(NPBF16)
    in_maps = []
    for b in range(B):
        m = dict(shared)
        m["x8"] = x8[b]
        m["attn8"] = attn8[b]
        m["attnT"] = attnTb[b]
        m["agT"] = agT[b]
        in_maps.append(m)
    return in_maps


def kernel(**inputs):
    zb = all(not np.any(np.asarray(inputs[k]))
             for k in ("bq_lf", "bk_ag", "bq_ag", "bk_hf", "bv_hf", "bproj",
                       "ba", "bb"))
    key = ("nc", zb)
    if key not in _CACHE:
        _CACHE[key] = _build_bass(zero_bias=zb)
    nc = _CACHE[key]
    in_maps = _prep_host(inputs)
    res = run_bass_kernel_spmd(nc, in_maps, core_ids=list(range(B)))
    return np.stack([res.results[b]["out"] for b in range(B)], axis=0)


# revision 31
# speedup vs baseline: 1.6280x; 1.0283x over previous
"""Trainium2 Bass kernel for the dual-branch agent-attention module.

Sharding: data-parallel over B=8 (one batch element per NeuronCore).
All transposes and weight permutations are done host-side; on-device
work is a streamed pipeline with the two big score contractions run in
fp8e4m3 DoubleRow mode (4 rows of K per PE pass -> 3x fewer tensor
cycles than bf16), which is safe here because the effective score
weights are tiny (|s| < 0.7) and the softmax is near-uniform.

Structure:
  prep:  agent projections k_ag/qa -> block-diagonal tiles; effective
         score weights Weff_A = Wq @ k12bd and Weff_B = Wkhf @ qabd
         (the big activations never materialize q or kh), written as
         fp8 scaled by 64 (undone via the exp activation's scale).
  AC-1:  branch-A score maps pa = exp(x @ Weff_A / 64) for all six
         head pairs, computed via fp8-DR matmuls; interleaved between
         early phase-B tiles so the scalar engine's exps overlap
         tensor work.
  B:     v = attnT^T@Wv (ones col = softmax denominator), tT scores
         from attn8 via Weff_B (fp8-DR) -> exp -> xs accumulated
         directly in a persistent PSUM bank across all 32 seq tiles.
  AC-2:  x_out = pa^T @ xs_bd with ones-column denominators,
         normalize, PE-transpose, proj.

Head-major layout trick: the 2C projection outputs are permuted host-
side from (branch, head, d) to (head, branch, d) so each head pair
occupies one 128-partition tile; branch score scales (wa/wb * D^-0.5)
are folded into the K-side weights, so both branches' score maps come
out of a single contraction per head pair.

Bias handling: k-side biases that are constant along a softmax axis
cancel exactly and are dropped (bk_hf entirely; q-side bias of branch
A survives as the per-agent term c_A = k12bd^T @ bq, applied as the
exp's per-partition bias together with ba).
"""

import os
import sys
import numpy as np

for _p in ("/opt/trn_rl_repo", os.path.expanduser("~/.axon_site/_ro/trn_rl_repo")):
    if os.path.isdir(_p) and _p not in sys.path:
        sys.path.insert(0, _p)

import ml_dtypes

import concourse.bass as bass
import concourse.bacc as bacc
import concourse.tile as tile
from concourse import mybir
from concourse.bass_utils import run_bass_kernel_spmd
from concourse.masks import make_identity

BF16 = mybir.dt.bfloat16
FP8 = mybir.dt.float8e4
F32 = mybir.dt.float32
NPBF16 = ml_dtypes.bfloat16
NPFP8 = ml_dtypes.float8_e4m3
DR = mybir.MatmulPerfMode.DoubleRow

B, N, NA, H, D = 8, 4096, 64, 12, 32
C = H * D            # 384
C2 = 2 * C           # 768
NP = H // 2          # 6 head pairs
NT = N // 128        # 32 seq tiles
SCALE = D ** -0.5
WSC = 64.0           # fp8 weight scale for the score contractions
ISC = 1.0 / WSC

_CACHE = {}


def _prune_ldweights(nc):
    """Drop InstLdweights whose weights AP matches the currently loaded one.

    The PE array keeps its stationary weights across matmuls; bass emits a
    fresh LDWEIGHTS per matmul regardless. Back-to-back matmuls that share
    lhsT (same AP, perf mode, tile geometry) only need the first load.
    Only bare loads (no semaphore waits/updates) are removed.
    """
    removed = 0
    for f in nc.m.functions:
        for bb in f.blocks:
            cur = None
            to_remove = []
            for inst in bb.instructions:
                tn = type(inst).__name__
                if getattr(inst, "engine", None) != mybir.EngineType.PE:
                    continue
                if tn == "InstLdweights":
                    ap = inst.ins[0]
                    sig = (ap.memref, ap.offset,
                           tuple(tuple(x) for x in ap.ap), str(ap.dtype),
                           str(getattr(inst, "perf_mode", None)),
                           bool(getattr(inst, "is_transpose", False)),
                           tuple(inst.tile_position) if inst.tile_position else None,
                           tuple(inst.tile_size) if inst.tile_size else None)
                    si = inst.sync_info
                    clean = si is None or (len(si.on_wait) == 0
                                           and len(si.on_update) == 0)
                    if cur == sig and clean:
                        to_remove.append(inst)
                    else:
                        cur = sig
                elif tn == "InstMatmult":
                    if getattr(inst, "is_transpose", False):
                        cur = None
                elif tn == "InstEventSemaphore":
                    pass
                else:
                    cur = None
            for inst in to_remove:
                bb.instructions.remove(inst)
                removed += 1
    return removed


def _build_bass(finalize=True, zero_bias=False, debug=False):
    nc = bacc.Bacc()

    # ---- DRAM I/O ----
    x8 = nc.dram_tensor("x8", [C, N], FP8, kind="ExternalInput")
    attn8 = nc.dram_tensor("attn8", [C, N], FP8, kind="ExternalInput")
    attnT = nc.dram_tensor("attnT", [C, N], BF16, kind="ExternalInput")
    agT = nc.dram_tensor("agT", [C, NA], BF16, kind="ExternalInput")
    wqT = nc.dram_tensor("wqT", [C2, C], BF16, kind="ExternalInput")
    wkag = nc.dram_tensor("wkag", [C, C2], BF16, kind="ExternalInput")
    wqag = nc.dram_tensor("wqag", [C, C2], BF16, kind="ExternalInput")
    wkhfT = nc.dram_tensor("wkhfT", [C2, C], BF16, kind="ExternalInput")
    wv = nc.dram_tensor("wv", [C, H * 33], BF16, kind="ExternalInput")
    wproj = nc.dram_tensor("wproj", [C, C], BF16, kind="ExternalInput")
    bq = nc.dram_tensor("bq", [C2], F32, kind="ExternalInput")
    bkag = nc.dram_tensor("bkag", [C2], F32, kind="ExternalInput")
    bqag = nc.dram_tensor("bqag", [C2], F32, kind="ExternalInput")
    bv = nc.dram_tensor("bv", [H * 33], F32, kind="ExternalInput")
    bproj = nc.dram_tensor("bproj", [C], F32, kind="ExternalInput")
    bab = nc.dram_tensor("bab", [2], F32, kind="ExternalInput")
    out = nc.dram_tensor("out", [N, C], F32, kind="ExternalOutput")
    if debug:
        d_pa = nc.dram_tensor("d_pa", [128, 6, N], BF16, kind="ExternalOutput")
        d_xsbd = nc.dram_tensor("d_xsbd", [128, 6 * 66], BF16, kind="ExternalOutput")
        d_pt0 = nc.dram_tensor("d_pt0", [128, C2], BF16, kind="ExternalOutput")
        d_v0 = nc.dram_tensor("d_v0", [128, H * 33], BF16, kind="ExternalOutput")
        d_weffa = nc.dram_tensor("d_weffa", [128, 4, C2], FP8, kind="ExternalOutput")
        d_weffb = nc.dram_tensor("d_weffb", [128, 4, C2], FP8, kind="ExternalOutput")
        d_pt5 = nc.dram_tensor("d_pt5", [128, C2], BF16, kind="ExternalOutput")
        d_v5 = nc.dram_tensor("d_v5", [128, H * 33], BF16, kind="ExternalOutput")
        d_xsraw = nc.dram_tensor("d_xsraw", [128, 6 * 66], F32, kind="ExternalOutput")

    Exp = mybir.ActivationFunctionType.Exp
    Copy = mybir.ActivationFunctionType.Copy

    with tile.TileContext(nc) as tc:
        with (
            tc.tile_pool(name="const", bufs=1) as const,
            tc.tile_pool(name="pt", bufs=2) as p_pt,
            tc.tile_pool(name="vv", bufs=2) as p_v,
            tc.tile_pool(name="xon", bufs=3) as p_xon,
            tc.tile_pool(name="xot", bufs=3) as p_xot,
            tc.tile_pool(name="osb", bufs=3) as p_out,
            tc.tile_pool(name="sm", bufs=4) as p_sm,
            tc.tile_pool(name="psA", bufs=2, space="PSUM") as psA,
            tc.tile_pool(name="psS", bufs=2, space="PSUM") as psS,
            tc.tile_pool(name="psV", bufs=1, space="PSUM") as psV,
            tc.tile_pool(name="psX", bufs=1, space="PSUM") as psX,
        ):
            # ---- weights: big ones on the fast HW DGE queues (sync/scalar);
            # only tiny constants ride the slow gpsimd sw queue
            ag_t = const.tile([128, 3, NA], BF16)
            nc.gpsimd.dma_start(out=ag_t, in_=agT.rearrange("(k p) m -> p k m", p=128))
            w_kag = const.tile([128, 3, C2], BF16)
            w_qag = const.tile([128, 3, C2], BF16)
            w_qT = const.tile([128, 6, C], BF16)
            w_khfT = const.tile([128, 6, C], BF16)
            w_v = const.tile([128, 3, H * 33], BF16)
            w_pr = const.tile([128, 3, C], BF16)
            for dst, src in ((w_kag, wkag), (w_qag, wqag)):
                nc.sync.dma_start(out=dst, in_=src.rearrange("(k p) m -> p k m", p=128))
            for dst, src in ((w_qT, wqT), (w_khfT, wkhfT), (w_v, wv)):
                nc.scalar.dma_start(out=dst, in_=src.rearrange("(k p) m -> p k m", p=128))
            b_q = const.tile([128, 6], F32)
            b_kag = const.tile([128, 6], F32)
            b_qag = const.tile([128, 6], F32)
            for dst, src in ((b_q, bq), (b_kag, bkag), (b_qag, bqag)):
                nc.gpsimd.dma_start(out=dst, in_=src.rearrange("(j p) -> p j", p=128))
            bv_row = const.tile([1, H * 33], BF16)
            nc.gpsimd.dma_start(out=bv_row, in_=bv[:].unsqueeze(0))
            bpr_row = const.tile([1, C], BF16)
            nc.gpsimd.dma_start(out=bpr_row, in_=bproj[:].unsqueeze(0))
            ones_row = const.tile([1, 128], BF16)
            nc.vector.memset(ones_row, 1.0)
            ba_t = const.tile([128, 1], F32)
            nc.gpsimd.dma_start(out=ba_t, in_=bass.AP(tensor=bab[:].tensor, offset=0,
                                                      ap=[[0, 128], [1, 1]]))
            bb_t = const.tile([128, 1], F32)
            nc.gpsimd.dma_start(out=bb_t, in_=bass.AP(tensor=bab[:].tensor, offset=1,
                                                      ap=[[0, 128], [1, 1]]))
            ident = const.tile([128, 128], BF16)
            make_identity(nc, ident)

            # ---- big input streams, split across sync + scalar DMA queues,
            # small leading pieces so the first tiles unblock early ----
            x8_t = const.tile([128, 4, N], FP8)
            at8_t = const.tile([128, 4, N], FP8)
            att_t = const.tile([128, 3, N], BF16)
            nc.vector.memset(x8_t[:, 3, :], 0.0)
            nc.vector.memset(at8_t[:, 3, :], 0.0)
            x8r = x8.rearrange("(k p) s -> p k s", p=128)
            at8r = attn8.rearrange("(k p) s -> p k s", p=128)
            attr = attnT.rearrange("(k p) s -> p k s", p=128)
            bounds = (0, 256, 512, 1024, 2048, 3072, 4096)
            for lo, hi in zip(bounds[:-1], bounds[1:]):
                sl = slice(lo, hi)
                nc.sync.dma_start(out=at8_t[:, 0:3, sl], in_=at8r[:, :, sl])
                nc.scalar.dma_start(out=att_t[:, :, sl], in_=attr[:, :, sl])
                nc.sync.dma_start(out=x8_t[:, 0:3, sl], in_=x8r[:, :, sl])
            nc.scalar.dma_start(out=w_pr, in_=wproj.rearrange("(k p) m -> p k m", p=128))

            # Pre-touch small DMA constants so wide consumers only carry
            # the producing engine's wait.
            touch = const.tile([128, 16], F32)
            for i, t_ap in enumerate((b_q[:, 0:1], b_kag[:, 0:1], b_qag[:, 0:1],
                                      ba_t[:, 0:1], bb_t[:, 0:1])):
                nc.vector.tensor_copy(touch[:, i:i + 1], t_ap)
            nc.scalar.copy(touch[:, 8:9], ba_t[:, 0:1])
            nc.scalar.copy(touch[:, 9:10], bb_t[:, 0:1])

            # ---- prep: k_ag / qa projections -> block-diag tiles ----
            kag_sb = const.tile([128, 6, NA], BF16)
            qa_sb = const.tile([128, 6, NA], BF16)
            for w_t, b_t, dst in ((w_kag, b_kag, kag_sb), (w_qag, b_qag, qa_sb)):
                for j in range(6):
                    ps = psA.tile([128, NA], F32, tag="a")
                    for k in range(3):
                        nc.tensor.matmul(ps, lhsT=w_t[:, k, j * 128:(j + 1) * 128],
                                         rhs=ag_t[:, k, :], start=(k == 0), stop=(k == 2))
                    nc.vector.tensor_add(dst[:, j, :], ps,
                                         b_t[:, j:j + 1].to_broadcast([128, NA]))
            k12bd = const.tile([128, 6, 128], BF16)
            qabd = const.tile([128, 6, 128], BF16)
            for src, dst in ((kag_sb, k12bd), (qa_sb, qabd)):
                nc.vector.memset(dst, 0.0)
                for j in range(6):
                    nc.vector.tensor_copy(dst[0:64, j, 0:64], src[0:64, j, :])
                    nc.vector.tensor_copy(dst[64:128, j, 64:128], src[64:128, j, :])

            # ---- prep: effective score weights (fp8, scaled by WSC) ----
            weff_a8 = const.tile([128, 4, C2], FP8)
            weff_b8 = const.tile([128, 4, C2], FP8)
            nc.vector.memset(weff_a8[:, 3, :], 0.0)
            nc.vector.memset(weff_b8[:, 3, :], 0.0)
            for j in range(6):
                for k in range(3):
                    ps = psA.tile([128, 128], F32, tag="a")
                    nc.tensor.matmul(ps, lhsT=w_qT[:, j, k * 128:(k + 1) * 128],
                                     rhs=k12bd[:, j, :], start=True, stop=True)
                    nc.scalar.activation(weff_a8[:, k, j * 128:(j + 1) * 128], ps,
                                         Copy, scale=WSC)
                    ps2 = psA.tile([128, 128], F32, tag="a")
                    nc.tensor.matmul(ps2, lhsT=w_khfT[:, j, k * 128:(k + 1) * 128],
                                     rhs=qabd[:, j, :], start=True, stop=True)
                    nc.scalar.activation(weff_b8[:, k, j * 128:(j + 1) * 128], ps2,
                                         Copy, scale=WSC)
            cba = None
            if not zero_bias:
                b_q_bf = const.tile([128, 6], BF16)
                nc.vector.tensor_copy(b_q_bf, b_q)
                cba = const.tile([128, 6], F32)
                for j in range(6):
                    ps = psA.tile([128, 1], F32, tag="a")
                    nc.tensor.matmul(ps, lhsT=k12bd[:, j, :], rhs=b_q_bf[:, j:j + 1],
                                     start=True, stop=True)
                    nc.vector.tensor_add(cba[:, j:j + 1], ps, ba_t[:, 0:1])

            # ---- interleaved AC-1 (branch-A score maps) + phase B ----
            pa_t = const.tile([128, 6, N], BF16)
            xs_ps = psX.tile([128, 6 * 66], F32, tag="x")

            def ac1_unit(u):
                # one head pair x two seq chunks; shared lhsT back-to-back so
                # the redundant LDWEIGHTS prune below can drop half the loads
                j, cp = u // 4, u % 4
                tiles = []
                for ci in (2 * cp, 2 * cp + 1):
                    csl = slice(ci * 512, (ci + 1) * 512)
                    ps = psA.tile([128, 512], F32, tag="a", name=f"ac1_{u}_{ci}")
                    tiles.append((ps, csl))
                for kp in (0, 1):
                    for ps, csl in tiles:
                        nc.tensor.matmul(ps,
                                         lhsT=weff_a8[:, 2 * kp:2 * kp + 2,
                                                      j * 128:(j + 1) * 128],
                                         rhs=x8_t[:, 2 * kp:2 * kp + 2, csl],
                                         start=(kp == 0), stop=(kp == 1),
                                         perf_mode=DR)
                abias = 0.0 if zero_bias else cba[:, j:j + 1]
                for ps, csl in tiles:
                    nc.scalar.activation(pa_t[:, j, csl], ps, Exp, bias=abias,
                                         scale=ISC)

            for t in range(NT):
                if t < 24:
                    ac1_unit(t)
                tsl = slice(t * 128, (t + 1) * 128)
                # branch-B scores for this seq tile (fp8 DoubleRow)
                ps_s = psS.tile([128, C2], F32, tag="s")
                for kp in (0, 1):
                    for r0, r1 in ((0, 512), (512, 768)):
                        nc.tensor.matmul(ps_s[:, r0:r1],
                                         lhsT=at8_t[:, 2 * kp:2 * kp + 2, tsl],
                                         rhs=weff_b8[:, 2 * kp:2 * kp + 2, r0:r1],
                                         start=(kp == 0), stop=(kp == 1),
                                         perf_mode=DR)
                pt = p_pt.tile([128, C2], BF16)
                bbias = 0.0 if zero_bias else bb_t[:, 0:1]
                nc.scalar.activation(pt, ps_s, Exp, bias=bbias, scale=ISC)
                # v for this seq tile
                psv = psV.tile([128, H * 33], F32, tag="v")
                for k in range(3):
                    nc.tensor.matmul(psv, lhsT=att_t[:, k, tsl], rhs=w_v[:, k, :],
                                     start=(k == 0), stop=zero_bias and (k == 2))
                v_t = p_v.tile([128, H * 33], BF16)
                v3 = v_t[:].rearrange("p (h c) -> p h c", c=33)
                if zero_bias:
                    pv3 = psv.rearrange("p (h c) -> p h c", c=33)
                    nc.vector.tensor_copy(v3[:, :, 0:32], pv3[:, :, 0:32])
                    nc.vector.memset(v3[:, :, 32], 1.0)
                else:
                    nc.tensor.matmul(psv, lhsT=ones_row[:, :], rhs=bv_row[:, :],
                                     start=False, stop=True)
                    nc.vector.tensor_copy(v_t, psv)
                # xs accumulation directly in PSUM across all tiles
                for j in range(6):
                    # start only on the very first write: start_tensor_calc
                    # marks the whole 2KB bank pending-zero, so each later
                    # region's first write overwrites, then accumulates.
                    nc.tensor.matmul(xs_ps[:, j * 66:(j + 1) * 66],
                                     lhsT=pt[:, j * 128:(j + 1) * 128],
                                     rhs=v_t[:, j * 66:(j + 1) * 66],
                                     start=(t == 0 and j == 0), stop=(t == NT - 1),
                                     skip_group_check=True)
                if debug and t == 0:
                    nc.sync.dma_start(out=d_pt0[:, :], in_=pt)
                    nc.sync.dma_start(out=d_v0[:, :], in_=v_t)
                if debug and t == 5:
                    nc.sync.dma_start(out=d_pt5[:, :], in_=pt)
                    nc.sync.dma_start(out=d_v5[:, :], in_=v_t)

            # ---- xs normalize -> block-diag [xs | 1] tiles ----
            xs_bd = const.tile([128, 6 * 66], BF16)
            xs3 = xs_ps.rearrange("p (j c) -> p j c", c=66)
            bd3 = xs_bd[:].rearrange("p (j c) -> p j c", c=66)
            nc.vector.memset(xs_bd, 0.0)
            nc.vector.memset(bd3[0:64, :, 32:33], 1.0)
            nc.vector.memset(bd3[64:128, :, 65:66], 1.0)
            rec6 = p_sm.tile([128, 6], F32, tag="rec")
            nc.vector.reciprocal(rec6[0:64, :], xs3[0:64, :, 32])
            nc.vector.reciprocal(rec6[64:128, :], xs3[64:128, :, 65])
            nc.vector.tensor_mul(bd3[0:64, :, 0:32], xs3[0:64, :, 0:32],
                                 rec6[0:64, :].unsqueeze(2).to_broadcast([64, 6, 32]))
            nc.vector.tensor_mul(bd3[64:128, :, 33:65], xs3[64:128, :, 33:65],
                                 rec6[64:128, :].unsqueeze(2).to_broadcast([64, 6, 32]))

            if debug:
                xsr_sb = const.tile([128, 6 * 66], F32)
                nc.scalar.copy(xsr_sb, xs_ps)
                nc.sync.dma_start(out=d_xsraw[:, :], in_=xsr_sb)
                nc.sync.dma_start(out=d_pa[:, :, :], in_=pa_t)
                nc.sync.dma_start(out=d_xsbd[:, :], in_=xs_bd)
                nc.sync.dma_start(out=d_weffa[:, :, :], in_=weff_a8)
                nc.sync.dma_start(out=d_weffb[:, :, :], in_=weff_b8)

            # ---- AC-2: x_out, normalize, transpose, proj ----
            for t in range(NT):
                tsl = slice(t * 128, (t + 1) * 128)
                xo_ps = psA.tile([128, 12 * 33], F32, tag="a", name=f"xo_{t}")
                for j in range(6):
                    nc.tensor.matmul(xo_ps[:, j * 66:(j + 1) * 66],
                                     lhsT=pa_t[:, j, tsl],
                                     rhs=xs_bd[:, j * 66:(j + 1) * 66],
                                     start=True, stop=True)
                xo3 = xo_ps.rearrange("p (k c) -> p k c", c=33)
                rec = p_sm.tile([128, 12], F32, tag="rec12")
                nc.vector.reciprocal(rec, xo3[:, :, 32])
                xon = p_xon.tile([128, C], BF16)
                nc.vector.tensor_mul(xon[:].rearrange("p (k c) -> p k c", c=32),
                                     xo3[:, :, 0:32],
                                     rec[:].unsqueeze(2).to_broadcast([128, 12, 32]))
                tp_ps = psA.tile([128, C], BF16, tag="a", name=f"tp_{t}")
                for f in range(3):
                    nc.tensor.transpose(tp_ps[:, f * 128:(f + 1) * 128],
                                        xon[:, f * 128:(f + 1) * 128], ident)
                xot = p_xot.tile([128, C], BF16)
                nc.vector.tensor_copy(xot, tp_ps)
                pr_ps = psS.tile([128, C], F32, tag="s", name=f"pr_{t}")
                for f in range(3):
                    nc.tensor.matmul(pr_ps, lhsT=xot[:, f * 128:(f + 1) * 128],
                                     rhs=w_pr[:, f, :],
                                     start=(f == 0),
                                     stop=(zero_bias and f == 2),
                                     skip_group_check=True)
                if not zero_bias:
                    nc.tensor.matmul(pr_ps, lhsT=ones_row[:, :], rhs=bpr_row[:, :],
                                     start=False, stop=True, skip_group_check=True)
                o_sb = p_out.tile([128, C], F32)
                nc.scalar.copy(o_sb, pr_ps)
                nc.sync.dma_start(out=out[t * 128:(t + 1) * 128, :], in_=o_sb)
    _prune_ldweights(nc)
    if finalize:
        nc.finalize()
    return nc


def _prep_host(inputs):
    f32 = np.float32
    x = np.asarray(inputs["x"], f32)
    attn = np.asarray(inputs["attn"], f32)
    agent = np.asarray(inputs["agent_input"], f32)
    wa = np.asarray(inputs["wa"], f32)
    wb = np.asarray(inputs["wb"], f32)

    perm = np.empty(C2, np.int64)
    sva = np.empty(C2, f32)
    svb = np.empty(C2, f32)
    for h in range(H):
        for br in range(2):
            j0 = h * 64 + br * 32
            perm[j0:j0 + 32] = br * C + h * 32 + np.arange(32)
            sva[j0:j0 + 32] = wa[br] * SCALE
            svb[j0:j0 + 32] = wb[br] * SCALE

    wq_p = np.asarray(inputs["Wq_lf"], f32)[:, perm]
    bq_p = np.asarray(inputs["bq_lf"], f32)[perm]
    wkag_p = np.asarray(inputs["Wk_ag"], f32)[:, perm] * sva[None, :]
    bkag_p = np.asarray(inputs["bk_ag"], f32)[perm] * sva
    wqag_p = np.asarray(inputs["Wq_ag"], f32)[:, perm]
    bqag_p = np.asarray(inputs["bq_ag"], f32)[perm]
    wkhf_p = np.asarray(inputs["Wk_hf"], f32)[:, perm] * svb[None, :]

    wv_in = np.asarray(inputs["Wv_hf"], f32)
    bv_in = np.asarray(inputs["bv_hf"], f32)
    wv_aug = np.zeros((C, H * 33), f32)
    bv_aug = np.zeros(H * 33, f32)
    for h in range(H):
        wv_aug[:, h * 33:h * 33 + 32] = wv_in[:, h * 32:h * 32 + 32]
        bv_aug[h * 33:h * 33 + 32] = bv_in[h * 32:h * 32 + 32]
        bv_aug[h * 33 + 32] = 1.0

    bab = np.array([np.asarray(inputs["ba"], f32)[0],
                    np.asarray(inputs["bb"], f32)[0]], f32)

    shared = {
        "wqT": np.ascontiguousarray(wq_p.T).astype(NPBF16),
        "wkhfT": np.ascontiguousarray(wkhf_p.T).astype(NPBF16),
        "wkag": wkag_p.astype(NPBF16),
        "wqag": wqag_p.astype(NPBF16),
        "wv": wv_aug.astype(NPBF16),
        "wproj": np.asarray(inputs["Wproj"], f32).astype(NPBF16),
        "bq": bq_p, "bkag": bkag_p, "bqag": bqag_p,
        "bv": bv_aug, "bproj": np.ascontiguousarray(np.asarray(inputs["bproj"], f32)),
        "bab": bab,
    }
    xT = np.ascontiguousarray(x.transpose(0, 2, 1))
    attnT = np.ascontiguousarray(attn.transpose(0, 2, 1))
    agT = np.ascontiguousarray(agent.transpose(0, 2, 1)).astype(NPBF16)
    x8 = xT.astype(NPFP8)
    attn8 = attnT.astype(NPFP8)
    attnTb = attnT.astype(NPBF16)
    in_maps = []
    for b in range(B):
        m = dict(shared)
        m["x8"] = x8[b]
        m["attn8"] = attn8[b]
        m["attnT"] = attnTb[b]
        m["agT"] = agT[b]
        in_maps.append(m)
    return in_maps


def kernel(**inputs):
    zb = all(not np.any(np.asarray(inputs[k]))
             for k in ("bq_lf", "bk_ag", "bq_ag", "bk_hf", "bv_hf", "bproj",
                       "ba", "bb"))
    key = ("nc", zb)
    if key not in _CACHE:
        _CACHE[key] = _build_bass(zero_bias=zb)
    nc = _CACHE[key]
    in_maps = _prep_host(inputs)
    res = run_bass_kernel_spmd(nc, in_maps, core_ids=list(range(B)))
    return np.stack([res.results[b]["out"] for b in range(B)], axis=0)
